# revision 1
# baseline (speedup 1.0000x reference)
"""Trainium2 Bass kernel for nn_Decoder (2-layer LSTM decoder, autoregressive).

Reference computation (per timestep t, batch B=1024):
  L0: gates = z @ W_ih0.T + b_ih0 + h0 @ W_hh0.T + b_hh0 ; i,f,g,o = split(gates)
      c0' = sig(f)*c0 + sig(i)*tanh(g) ; h0' = sig(o)*tanh(c0')
  L1: same with h0' as input
  z' = h1' @ fc_W.T + fc_b          (autoregressive feedback)
  out[t] = z' @ lin_W.T + lin_b

Sharding: data-parallel over batch, 8 cores x 128 batch each; weights
replicated and resident in SBUF; the time loop is fully unrolled on-device.

Layout strategy (per core, B=128):
  - Gate matmuls run batch-major: PSUM[batch=128, gates<=512] with the
    activation (feature-major [feat,128]) as the stationary lhsT and the
    transposed weight [feat, 4H] as the N=512 moving operand, in float32r
    (full PE rate at N>=256, ~1e-4 matmul rel error).
  - Biases are added via K=1 rank-1 matmuls (ones[1,128] x bias[1,N]).
  - sigmoid/tanh on ScalarE straight out of PSUM; products on VectorE.
  - h'/z' come out batch-major; PE transposes (through an identity matmul)
    rebuild the feature-major stationaries for the next step.
"""

import sys

sys.path.insert(0, "/opt/trn_rl_repo")

import ml_dtypes
import numpy as np

import concourse.bass as bass
from concourse import bacc, mybir
from concourse.tile import TileContext
from concourse.bass_utils import run_bass_kernel_spmd
from concourse.masks import make_identity

F32 = mybir.dt.float32
F32R = mybir.dt.float32r
AF = mybir.ActivationFunctionType

INPUT, HIDDEN, OUTPUT = 256, 512, 256
H4 = 4 * HIDDEN
B_LOCAL = 128
N_CORES = 8
P = 128
KX0 = INPUT // P   # 2  z feature chunks
KH = HIDDEN // P   # 4  h feature chunks
GATE_ORDER = (1, 0, 2, 3)  # emit f first (the c-chain needs sig(f) first), then i, g, o


YB = 4  # output steps batched per DMA


def build(T=128, batch_dma=True, fc_direct=True, rep=1, dve_bias=False,
          bias_rowtile=True):
    nc = bacc.Bacc("TRN2", target_bir_lowering=False, debug=False, num_devices=N_CORES)

    zT_p = nc.declare_dram_parameter("zT0", [INPUT, B_LOCAL], F32R, isOutput=False)
    h0T_p = nc.declare_dram_parameter("h0T_l0", [HIDDEN, B_LOCAL], F32R, isOutput=False)
    h1T_p = nc.declare_dram_parameter("h0T_l1", [HIDDEN, B_LOCAL], F32R, isOutput=False)
    c0_p = nc.declare_dram_parameter("c_l0", [B_LOCAL, HIDDEN], F32, isOutput=False)
    c1_p = nc.declare_dram_parameter("c_l1", [B_LOCAL, HIDDEN], F32, isOutput=False)
    w0x_p = nc.declare_dram_parameter("w0x", [INPUT, H4], F32R, isOutput=False)
    w0h_p = nc.declare_dram_parameter("w0h", [HIDDEN, H4], F32R, isOutput=False)
    w1x_p = nc.declare_dram_parameter("w1x", [HIDDEN, H4], F32R, isOutput=False)
    w1h_p = nc.declare_dram_parameter("w1h", [HIDDEN, H4], F32R, isOutput=False)
    wfc_p = nc.declare_dram_parameter("wfc", [HIDDEN, INPUT], F32R, isOutput=False)
    wlin_p = nc.declare_dram_parameter("wlin", [INPUT, OUTPUT], F32R, isOutput=False)
    b0_p = nc.declare_dram_parameter("bias0", [1, H4], F32R, isOutput=False)
    b1_p = nc.declare_dram_parameter("bias1", [1, H4], F32R, isOutput=False)
    bfc_p = nc.declare_dram_parameter("bfc", [1, INPUT], F32R, isOutput=False)
    blin_p = nc.declare_dram_parameter("blin", [1, OUTPUT], F32R, isOutput=False)
    bfcT_p = nc.declare_dram_parameter("bfcT", [P, KX0], F32, isOutput=False)
    ones_p = nc.declare_dram_parameter("ones", [1, B_LOCAL], F32R, isOutput=False)
    BF16 = mybir.dt.bfloat16
    b0r_p = nc.declare_dram_parameter("b0r", [P, HIDDEN], BF16, isOutput=False)
    b1r_p = nc.declare_dram_parameter("b1r", [P, HIDDEN], BF16, isOutput=False)
    onesf_p = nc.declare_dram_parameter("onesf", [P, B_LOCAL], BF16, isOutput=False)
    y_p = nc.declare_dram_parameter("y", [T, B_LOCAL, OUTPUT], F32, isOutput=True)
    y_ap = y_p[:]

    with TileContext(nc) as tc:
        with (
            tc.tile_pool(name="wpool", bufs=1) as wp,
            tc.tile_pool(name="state", bufs=2) as sp,
            tc.tile_pool(name="work", bufs=2) as wk,
            tc.tile_pool(name="gpsum", bufs=4, space="PSUM") as gp,
            tc.tile_pool(name="spsum", bufs=4, space="PSUM") as pp,
        ):
            # ---- one-time loads: weights, biases, identity, initial state ----
            w0x = wp.tile([P, KX0, H4], F32R, tag="w0x")
            w0h = wp.tile([P, KH, H4], F32R, tag="w0h")
            w1x = wp.tile([P, KH, H4], F32R, tag="w1x")
            w1h = wp.tile([P, KH, H4], F32R, tag="w1h")
            wfc = wp.tile([P, KH, INPUT], F32R, tag="wfc")
            wlin = wp.tile([P, KX0, OUTPUT], F32R, tag="wlin")
            nc.sync.dma_start(w0x[:], w0x_p[:].rearrange("(kc p) n -> p kc n", p=P))
            nc.sync.dma_start(w0h[:], w0h_p[:].rearrange("(kc p) n -> p kc n", p=P))
            nc.sync.dma_start(w1x[:], w1x_p[:].rearrange("(kc p) n -> p kc n", p=P))
            nc.sync.dma_start(w1h[:], w1h_p[:].rearrange("(kc p) n -> p kc n", p=P))
            nc.sync.dma_start(wfc[:], wfc_p[:].rearrange("(kc p) n -> p kc n", p=P))
            nc.sync.dma_start(wlin[:], wlin_p[:].rearrange("(kc p) n -> p kc n", p=P))

            bfcT = wp.tile([P, KX0], F32, tag="bfcT")
            nc.sync.dma_start(bfcT[:], bfcT_p[:])

            if dve_bias:
                # Broadcast biases to [P, ...] tiles once via rank-1 matmuls;
                # the [1, N] sources live in a preamble-scoped pool.
                b0b = wp.tile([P, 4, HIDDEN], mybir.dt.bfloat16, tag="b0b")
                b1b = wp.tile([P, 4, HIDDEN], mybir.dt.bfloat16, tag="b1b")
                blinb = wp.tile([P, OUTPUT], F32, tag="blinb")
                with tc.tile_pool(name="pre", bufs=1) as pre:
                    b0 = pre.tile([1, H4], F32R, tag="b0")
                    b1 = pre.tile([1, H4], F32R, tag="b1")
                    blin = pre.tile([1, OUTPUT], F32R, tag="blin")
                    ones = pre.tile([1, B_LOCAL], F32R, tag="ones")
                    nc.sync.dma_start(b0[:], b0_p[:])
                    nc.sync.dma_start(b1[:], b1_p[:])
                    nc.sync.dma_start(blin[:], blin_p[:])
                    nc.sync.dma_start(ones[:], ones_p[:])
                    for l, (bsrc, bdst) in enumerate(((b0, b0b), (b1, b1b))):
                        for g in range(4):
                            pb = pp.tile([P, HIDDEN], F32, tag="tr", name=f"pb_{l}_{g}")
                            nc.tensor.matmul(pb[:], ones[:],
                                             bsrc[:, g * HIDDEN:(g + 1) * HIDDEN],
                                             start=True, stop=True)
                            nc.scalar.activation(bdst[:, g], pb[:], AF.Copy)
                    pl = pp.tile([P, OUTPUT], F32, tag="tr", name="pl_bias")
                    nc.tensor.matmul(pl[:], ones[:], blin[:], start=True, stop=True)
                    nc.scalar.activation(blinb[:], pl[:], AF.Copy)
                b0 = b1 = blin = ones = None
            else:
                b0 = wp.tile([1, H4], F32R, tag="b0")
                b1 = wp.tile([1, H4], F32R, tag="b1")
                blin = wp.tile([1, OUTPUT], F32R, tag="blin")
                ones = wp.tile([1, B_LOCAL], F32R, tag="ones")
                nc.sync.dma_start(b0[:], b0_p[:])
                nc.sync.dma_start(b1[:], b1_p[:])
                nc.sync.dma_start(blin[:], blin_p[:])
                nc.sync.dma_start(ones[:], ones_p[:])
                b0b = b1b = blinb = None
            bfc = wp.tile([1, INPUT], F32R, tag="bfc")
            nc.sync.dma_start(bfc[:], bfc_p[:])
            if bias_rowtile:
                b0r = wp.tile([P, HIDDEN], BF16, tag="b0r")
                b1r = wp.tile([P, HIDDEN], BF16, tag="b1r")
                onesf = wp.tile([P, B_LOCAL], BF16, tag="onesf")
                nc.sync.dma_start(b0r[:], b0r_p[:])
                nc.sync.dma_start(b1r[:], b1r_p[:])
                nc.sync.dma_start(onesf[:], onesf_p[:])
            else:
                b0r = b1r = onesf = None
            brow = {"l0": b0r, "l1": b1r}

            ident = wp.tile([P, P], F32, tag="ident")
            make_identity(nc, ident[:])

            zT = wp.tile([P, KX0, B_LOCAL], F32R, tag="zT_init")
            h0T = wp.tile([P, KH, B_LOCAL], F32R, tag="h0T_init")
            h1T = wp.tile([P, KH, B_LOCAL], F32R, tag="h1T_init")
            c0 = wp.tile([P, HIDDEN], F32, tag="c0_init")
            c1 = wp.tile([P, HIDDEN], F32, tag="c1_init")
            nc.sync.dma_start(zT[:], zT_p[:].rearrange("(kc p) b -> p kc b", p=P))
            nc.sync.dma_start(h0T[:], h0T_p[:].rearrange("(kc p) b -> p kc b", p=P))
            nc.sync.dma_start(h1T[:], h1T_p[:].rearrange("(kc p) b -> p kc b", p=P))
            nc.sync.dma_start(c0[:], c0_p[:])
            nc.sync.dma_start(c1[:], c1_p[:])

            def lstm_layer(t, lname, xT, nx, hT, c, wx, wh, bias, bias_bcast):
                """One LSTM layer step. xT: [P, nx, B] stationary input chunks,
                hT: [P, KH, B], c: [P, HIDDEN]. Returns (h_batchmajor, hT_new, c_new)."""
                banks = {}
                for j, g in enumerate(GATE_ORDER):
                    ps = gp.tile([P, HIDDEN], F32, tag="gb", name=f"g_{lname}_{t}_{g}")
                    sl = slice(g * HIDDEN, (g + 1) * HIDDEN)
                    if bias_rowtile:
                        # K=1 rank-1 bias matmuls; the 4 banks' MMs target
                        # distinct 32-row groups so they run concurrently
                        nc.tensor.matmul(ps[:], onesf[32 * j:32 * j + 1, :],
                                         brow[lname][32 * j:32 * j + 1, :],
                                         start=True, stop=False,
                                         tile_position=(32 * j, 0))
                    elif not dve_bias:
                        nc.tensor.matmul(ps[:], ones[:], bias[:, sl], start=True, stop=False)
                    for k in range(KH):
                        nc.tensor.matmul(ps[:], hT[:, k], wh[:, k, sl],
                                         start=(dve_bias and k == 0), stop=False)
                    for k in range(nx):
                        nc.tensor.matmul(ps[:], xT[:, k], wx[:, k, sl],
                                         start=False, stop=(k == nx - 1))
                    banks[g] = ps

                sf = wk.tile([P, HIDDEN], F32, tag="sf", name=f"sf_{lname}_{t}")
                si = wk.tile([P, HIDDEN], F32, tag="si", name=f"si_{lname}_{t}")
                tg = wk.tile([P, HIDDEN], F32, tag="tg", name=f"tg_{lname}_{t}")
                so = wk.tile([P, HIDDEN], F32, tag="so", name=f"so_{lname}_{t}")
                if dve_bias:
                    nc.vector.tensor_add(out=sf[:], in0=banks[1][:], in1=bias_bcast[:, 1])
                    nc.vector.tensor_add(out=si[:], in0=banks[0][:], in1=bias_bcast[:, 0])
                    nc.vector.tensor_add(out=tg[:], in0=banks[2][:], in1=bias_bcast[:, 2])
                    nc.vector.tensor_add(out=so[:], in0=banks[3][:], in1=bias_bcast[:, 3])
                    nc.scalar.activation(sf[:], sf[:], AF.Sigmoid)
                    nc.scalar.activation(si[:], si[:], AF.Sigmoid)
                    nc.scalar.activation(tg[:], tg[:], AF.Tanh)
                    nc.scalar.activation(so[:], so[:], AF.Sigmoid)
                else:
                    nc.scalar.activation(sf[:], banks[1][:], AF.Sigmoid)
                    nc.scalar.activation(si[:], banks[0][:], AF.Sigmoid)
                    nc.scalar.activation(tg[:], banks[2][:], AF.Tanh)
                    nc.scalar.activation(so[:], banks[3][:], AF.Sigmoid)

                cn = sp.tile([P, HIDDEN], F32, tag=f"c_{lname}", name=f"c_{lname}_{t}")
                nc.vector.tensor_mul(out=sf[:], in0=sf[:], in1=c[:])
                nc.vector.tensor_mul(out=si[:], in0=si[:], in1=tg[:])
                nc.vector.tensor_add(out=cn[:], in0=sf[:], in1=si[:])
                nc.scalar.activation(tg[:], cn[:], AF.Tanh)
                hb = wk.tile([P, HIDDEN], F32, tag="hb", name=f"hb_{lname}_{t}")
                nc.vector.tensor_mul(out=hb[:], in0=so[:], in1=tg[:])

                ptr = pp.tile([P, KH, P], F32, tag="tr", name=f"htr_{lname}_{t}")
                for k in range(KH):
                    nc.tensor.transpose(ptr[:, k], hb[:, k * P:(k + 1) * P], ident[:])
                hTn = sp.tile([P, KH, B_LOCAL], F32R, tag=f"hT_{lname}", name=f"hT_{lname}_{t}")
                nc.vector.tensor_copy(out=hTn[:], in_=ptr[:])
                return hb, hTn, cn

            zT0, h0T0, h1T0, c00, c10 = zT, h0T, h1T, c0, c1

            def time_loop():
                ybuf = None
                zT, h0T, h1T, c0, c1 = zT0, h0T0, h1T0, c00, c10
                for t in range(T):
                    _, h0Tn, c0n = lstm_layer(t, "l0", zT, KX0, h0T, c0, w0x, w0h, b0, b0b)
                    _, h1Tn, c1n = lstm_layer(t, "l1", h0Tn, KH, h1T, c1, w1x, w1h, b1, b1b)

                    # fc: z' = h1' @ fc_W.T + fc_b
                    zTn = sp.tile([P, KX0, B_LOCAL], F32R, tag="zT", name=f"zT_{t}")
                    if fc_direct:
                        # feature-major out [z_feat, B]: stationary fc_W.T chunks,
                        # moving h1T; bias fused per-partition into the copyback
                        pzt = pp.tile([P, KX0, P], F32, tag="tr", name=f"fcT_{t}")
                        for k2 in range(KX0):
                            for k in range(KH):
                                nc.tensor.matmul(pzt[:, k2],
                                                 wfc[:, k, k2 * P:(k2 + 1) * P],
                                                 h1Tn[:, k],
                                                 start=(k == 0), stop=(k == KH - 1))
                        for k2 in range(KX0):
                            nc.scalar.activation(zTn[:, k2], pzt[:, k2], AF.Identity,
                                                 bias=bfcT[:, k2:k2 + 1])
                    else:
                        pfc = pp.tile([P, INPUT], F32, tag="tr", name=f"fc_{t}")
                        nc.tensor.matmul(pfc[:], ones[:], bfc[:], start=True, stop=False)
                        for k in range(KH):
                            nc.tensor.matmul(pfc[:], h1Tn[:, k], wfc[:, k],
                                             start=False, stop=(k == KH - 1))
                        zb = wk.tile([P, INPUT], F32, tag="zb", name=f"zb_{t}")
                        nc.scalar.activation(zb[:], pfc[:], AF.Copy)
                        pzt = pp.tile([P, KX0, P], F32, tag="tr", name=f"ztr_{t}")
                        for k in range(KX0):
                            nc.tensor.transpose(pzt[:, k], zb[:, k * P:(k + 1) * P], ident[:])
                        nc.vector.tensor_copy(out=zTn[:], in_=pzt[:])

                    # lin: out[t] = z' @ lin_W.T + lin_b
                    plin = pp.tile([P, OUTPUT], F32, tag="tr", name=f"lin_{t}")
                    if not dve_bias:
                        nc.tensor.matmul(plin[:], ones[:], blin[:], start=True, stop=False)
                    for k in range(KX0):
                        nc.tensor.matmul(plin[:], zTn[:, k], wlin[:, k],
                                         start=(dve_bias and k == 0),
                                         stop=(k == KX0 - 1))
                    if batch_dma:
                        if t % YB == 0:
                            ybuf = wk.tile([P, YB, OUTPUT], F32, tag="ybuf",
                                           name=f"ybuf_{t}")
                        if dve_bias:
                            nc.vector.tensor_add(out=ybuf[:, t % YB], in0=plin[:],
                                                 in1=blinb[:])
                        else:
                            nc.scalar.activation(ybuf[:, t % YB], plin[:], AF.Copy)
                        if t % YB == YB - 1 or t == T - 1:
                            n = t % YB + 1
                            nc.gpsimd.dma_start(
                                y_ap[t - n + 1:t + 1].rearrange("t b f -> b t f"),
                                ybuf[:, :n])
                    else:
                        yb = wk.tile([P, OUTPUT], F32, tag="yb", name=f"yb_{t}")
                        nc.scalar.activation(yb[:], plin[:], AF.Copy)
                        nc.sync.dma_start(y_ap[t], yb[:])

                    zT, h0T, h1T, c0, c1 = zTn, h0Tn, h1Tn, c0n, c1n

            if rep == 1:
                time_loop()
            else:
                with tc.For_i(0, rep, 1):
                    time_loop()

    nc.compile()
    return nc


def _bias_rows(b):
    """[P, HIDDEN] bf16: row 32*j holds the bias slice for gate GATE_ORDER[j]."""
    out = np.zeros((P, HIDDEN), dtype=ml_dtypes.bfloat16)
    for j, g in enumerate(GATE_ORDER):
        out[32 * j] = b[g * HIDDEN:(g + 1) * HIDDEN].astype(ml_dtypes.bfloat16)
    return out


def make_in_maps(z0, h0, c0, W_ih0, W_hh0, b_ih0, b_hh0,
                 W_ih1, W_hh1, b_ih1, b_hh1, fc_W, fc_b, lin_W, lin_b):
    f = np.float32
    shared = {
        "w0x": np.ascontiguousarray(W_ih0.T, dtype=f),
        "w0h": np.ascontiguousarray(W_hh0.T, dtype=f),
        "w1x": np.ascontiguousarray(W_ih1.T, dtype=f),
        "w1h": np.ascontiguousarray(W_hh1.T, dtype=f),
        "wfc": np.ascontiguousarray(fc_W.T, dtype=f),
        "wlin": np.ascontiguousarray(lin_W.T, dtype=f),
        "bias0": (b_ih0 + b_hh0).astype(f).reshape(1, H4),
        "bias1": (b_ih1 + b_hh1).astype(f).reshape(1, H4),
        "bfc": fc_b.astype(f).reshape(1, INPUT),
        "blin": lin_b.astype(f).reshape(1, OUTPUT),
        "bfcT": np.ascontiguousarray(fc_b.astype(f).reshape(KX0, P).T),
        "b0r": _bias_rows(b_ih0 + b_hh0),
        "b1r": _bias_rows(b_ih1 + b_hh1),
        "onesf": np.ones((P, B_LOCAL), dtype=ml_dtypes.bfloat16),
        "ones": np.ones((1, B_LOCAL), dtype=f),
    }
    in_maps = []
    for cidx in range(N_CORES):
        sl = slice(cidx * B_LOCAL, (cidx + 1) * B_LOCAL)
        in_maps.append({
            "zT0": np.ascontiguousarray(z0[sl].T, dtype=f),
            "h0T_l0": np.ascontiguousarray(h0[0, sl].T, dtype=f),
            "h0T_l1": np.ascontiguousarray(h0[1, sl].T, dtype=f),
            "c_l0": np.ascontiguousarray(c0[0, sl], dtype=f),
            "c_l1": np.ascontiguousarray(c0[1, sl], dtype=f),
            **shared,
        })
    return in_maps


_NC_CACHE = {}


def kernel(z0, h0, c0, W_ih0, W_hh0, b_ih0, b_hh0,
           W_ih1, W_hh1, b_ih1, b_hh1, fc_W, fc_b, lin_W, lin_b, T2):
    T = int(T2)
    if T not in _NC_CACHE:
        _NC_CACHE[T] = build(T)
    nc = _NC_CACHE[T]
    in_maps = make_in_maps(z0, h0, c0, W_ih0, W_hh0, b_ih0, b_hh0,
                           W_ih1, W_hh1, b_ih1, b_hh1, fc_W, fc_b, lin_W, lin_b)
    res = run_bass_kernel_spmd(nc, in_maps, list(range(N_CORES)))
    # per-core y: [T, 128, OUTPUT] -> full [T, 1024, OUTPUT]
    return np.concatenate([r["y"] for r in res.results], axis=1)



# revision 6
# speedup vs baseline: 1.0990x; 1.0990x over previous
"""Trainium2 Bass kernel for nn_Decoder (2-layer LSTM decoder, autoregressive).

Reference computation (per timestep t, batch B=1024):
  L0: gates = z @ W_ih0.T + b_ih0 + h0 @ W_hh0.T + b_hh0 ; i,f,g,o = split(gates)
      c0' = sig(f)*c0 + sig(i)*tanh(g) ; h0' = sig(o)*tanh(c0')
  L1: same with h0' as input
  z' = h1' @ fc_W.T + fc_b          (autoregressive feedback)
  out[t] = z' @ lin_W.T + lin_b

Sharding: data-parallel over batch, 8 cores x 128 batch each; weights
replicated and resident in SBUF; the time loop is fully unrolled on-device.

v2 layout strategy (per core, B=128), all matmul operands bf16 (PSUM f32):
  - Gate matmuls batch-major: PSUM[batch=128, gates=512] per gate bank, with
    the activation (feature-major [feat,128] bf16) stationary and the
    transposed weight [feat, 4H] bf16 moving (1 cycle/row at any N).
  - Gate biases are NOT matmuls: each bank's PSUM is preloaded with a
    broadcast bias tile by ScalarE/VectorE (off the critical path), and all
    gate matmuls accumulate with start=False.
  - sigmoid/tanh on ScalarE from PSUM; c-chain products on VectorE (f32).
  - h' is written bf16; PE transposes (bf16 identity, 1 cycle/row) rebuild
    the feature-major stationaries/moving operands for the next step.
  - fc emits z' feature-major with its bias fused per-partition into the
    ScalarE copyback; lin bias+copyback fused into one VectorE tensor_add.
"""

import sys

sys.path.insert(0, "/opt/trn_rl_repo")

import ml_dtypes
import numpy as np

import concourse.bass as bass
from concourse import bacc, mybir
from concourse.tile import TileContext
from concourse.bass_utils import run_bass_kernel_spmd
from concourse.masks import make_identity

F32 = mybir.dt.float32
F32R = mybir.dt.float32r
BF16 = mybir.dt.bfloat16
AF = mybir.ActivationFunctionType

INPUT, HIDDEN, OUTPUT = 256, 512, 256
H4 = 4 * HIDDEN
B_LOCAL = 128
N_CORES = 8
P = 128
KX0 = INPUT // P   # 2  z feature chunks
KH = HIDDEN // P   # 4  h feature chunks
GATE_ORDER = (1, 0, 2, 3)  # emit f first (the c-chain needs sig(f) first), then i, g, o

YB = 4  # output steps batched per DMA


def build(T=128, rep=1):
    nc = bacc.Bacc("TRN2", target_bir_lowering=False, debug=False, num_devices=N_CORES)

    zT_p = nc.declare_dram_parameter("zT0", [INPUT, B_LOCAL], BF16, isOutput=False)
    h0T_p = nc.declare_dram_parameter("h0T_l0", [HIDDEN, B_LOCAL], BF16, isOutput=False)
    h1T_p = nc.declare_dram_parameter("h0T_l1", [HIDDEN, B_LOCAL], BF16, isOutput=False)
    c0_p = nc.declare_dram_parameter("c_l0", [B_LOCAL, HIDDEN], F32, isOutput=False)
    c1_p = nc.declare_dram_parameter("c_l1", [B_LOCAL, HIDDEN], F32, isOutput=False)
    w0x_p = nc.declare_dram_parameter("w0x", [INPUT, H4], BF16, isOutput=False)
    w0h_p = nc.declare_dram_parameter("w0h", [HIDDEN, H4], BF16, isOutput=False)
    w1x_p = nc.declare_dram_parameter("w1x", [HIDDEN, H4], BF16, isOutput=False)
    w1h_p = nc.declare_dram_parameter("w1h", [HIDDEN, H4], BF16, isOutput=False)
    wfc_p = nc.declare_dram_parameter("wfc", [HIDDEN, INPUT], BF16, isOutput=False)
    wlin_p = nc.declare_dram_parameter("wlin", [INPUT, OUTPUT], BF16, isOutput=False)
    b0_p = nc.declare_dram_parameter("bias0", [1, H4], F32R, isOutput=False)
    b1_p = nc.declare_dram_parameter("bias1", [1, H4], F32R, isOutput=False)
    blin_p = nc.declare_dram_parameter("blin", [1, OUTPUT], F32R, isOutput=False)
    bfcT_p = nc.declare_dram_parameter("bfcT", [P, KX0], F32, isOutput=False)
    ones_p = nc.declare_dram_parameter("ones", [1, B_LOCAL], F32R, isOutput=False)
    y_p = nc.declare_dram_parameter("y", [T, B_LOCAL, OUTPUT], F32, isOutput=True)
    y_ap = y_p[:]

    with TileContext(nc) as tc:
        with (
            tc.tile_pool(name="wpool", bufs=1) as wp,
            tc.tile_pool(name="state", bufs=2) as sp,
            tc.tile_pool(name="work", bufs=2) as wk,
            tc.tile_pool(name="gpsum", bufs=4, space="PSUM") as gp,
            tc.tile_pool(name="spsum", bufs=4, space="PSUM") as pp,
        ):
            # ---- one-time loads: weights, biases, identity, initial state ----
            w0x = wp.tile([P, KX0, H4], BF16, tag="w0x")
            w0h = wp.tile([P, KH, H4], BF16, tag="w0h")
            w1x = wp.tile([P, KH, H4], BF16, tag="w1x")
            w1h = wp.tile([P, KH, H4], BF16, tag="w1h")
            wfc = wp.tile([P, KH, INPUT], BF16, tag="wfc")
            wlin = wp.tile([P, KX0, OUTPUT], BF16, tag="wlin")
            nc.sync.dma_start(w0x[:], w0x_p[:].rearrange("(kc p) n -> p kc n", p=P))
            nc.sync.dma_start(w0h[:], w0h_p[:].rearrange("(kc p) n -> p kc n", p=P))
            nc.sync.dma_start(w1x[:], w1x_p[:].rearrange("(kc p) n -> p kc n", p=P))
            nc.sync.dma_start(w1h[:], w1h_p[:].rearrange("(kc p) n -> p kc n", p=P))
            nc.sync.dma_start(wfc[:], wfc_p[:].rearrange("(kc p) n -> p kc n", p=P))
            nc.sync.dma_start(wlin[:], wlin_p[:].rearrange("(kc p) n -> p kc n", p=P))

            bfcT = wp.tile([P, KX0], F32, tag="bfcT")
            nc.sync.dma_start(bfcT[:], bfcT_p[:])

            # Broadcast biases to [P, ...] tiles once via rank-1 matmuls; the
            # [1, N] sources and the rank-1 PSUM live in preamble-scoped pools.
            b0b = wp.tile([P, 4, HIDDEN], F32, tag="b0b")
            b1b = wp.tile([P, 4, HIDDEN], F32, tag="b1b")
            blinb = wp.tile([P, OUTPUT], F32, tag="blinb")
            with tc.tile_pool(name="pre", bufs=1) as pre:
                b0 = pre.tile([1, H4], F32R, tag="b0")
                b1 = pre.tile([1, H4], F32R, tag="b1")
                blin = pre.tile([1, OUTPUT], F32R, tag="blin")
                ones = pre.tile([1, B_LOCAL], F32R, tag="ones")
                nc.sync.dma_start(b0[:], b0_p[:])
                nc.sync.dma_start(b1[:], b1_p[:])
                nc.sync.dma_start(blin[:], blin_p[:])
                nc.sync.dma_start(ones[:], ones_p[:])
                for l, (bsrc, bdst) in enumerate(((b0, b0b), (b1, b1b))):
                    for g in range(4):
                        pb = gp.tile([P, HIDDEN], F32, tag="gb", name=f"pb_{l}_{g}")
                        nc.tensor.matmul(pb[:], ones[:],
                                         bsrc[:, g * HIDDEN:(g + 1) * HIDDEN],
                                         start=True, stop=True)
                        nc.scalar.activation(bdst[:, g], pb[:], AF.Copy)
                pl = gp.tile([P, OUTPUT], F32, tag="gb", name="pl_bias")
                nc.tensor.matmul(pl[:], ones[:], blin[:], start=True, stop=True)
                nc.scalar.activation(blinb[:], pl[:], AF.Copy)

            ident = wp.tile([P, P], BF16, tag="ident")
            make_identity(nc, ident[:])

            zT = wp.tile([P, KX0, B_LOCAL], BF16, tag="zT_init")
            h0T = wp.tile([P, KH, B_LOCAL], BF16, tag="h0T_init")
            h1T = wp.tile([P, KH, B_LOCAL], BF16, tag="h1T_init")
            c0 = wp.tile([P, HIDDEN], F32, tag="c0_init")
            c1 = wp.tile([P, HIDDEN], F32, tag="c1_init")
            nc.sync.dma_start(zT[:], zT_p[:].rearrange("(kc p) b -> p kc b", p=P))
            nc.sync.dma_start(h0T[:], h0T_p[:].rearrange("(kc p) b -> p kc b", p=P))
            nc.sync.dma_start(h1T[:], h1T_p[:].rearrange("(kc p) b -> p kc b", p=P))
            nc.sync.dma_start(c0[:], c0_p[:])
            nc.sync.dma_start(c1[:], c1_p[:])

            def lstm_layer(t, lname, xT, nx, hT, c, wx, wh, bb):
                """One LSTM layer step. xT: [P, nx, B] bf16 stationary input
                chunks, hT: [P, KH, B] bf16, c: [P, HIDDEN] f32.
                Returns (hT_new, c_new)."""
                banks = {}
                # PSUM bias preload (off critical path): 2 banks via ScalarE,
                # 2 via VectorE. Gate matmuls then accumulate (start=False).
                for j, g in enumerate(GATE_ORDER):
                    ps = gp.tile([P, HIDDEN], F32, tag="gb", name=f"g_{lname}_{t}_{g}")
                    if j % 2 == 0:
                        nc.scalar.activation(ps[:], bb[:, g], AF.Copy)
                    else:
                        nc.vector.tensor_copy(out=ps[:], in_=bb[:, g])
                    banks[g] = ps
                # recurrent (h) parts first: ready as soon as last step's hT is
                for j, g in enumerate(GATE_ORDER):
                    ps = banks[g]
                    sl = slice(g * HIDDEN, (g + 1) * HIDDEN)
                    for k in range(KH):
                        nc.tensor.matmul(ps[:], hT[:, k], wh[:, k, sl],
                                         start=False, stop=False,
                                         skip_group_check=True)
                # input (x/z) parts second: they wait on the previous stage
                for j, g in enumerate(GATE_ORDER):
                    ps = banks[g]
                    sl = slice(g * HIDDEN, (g + 1) * HIDDEN)
                    for k in range(nx):
                        nc.tensor.matmul(ps[:], xT[:, k], wx[:, k, sl],
                                         start=False, stop=(k == nx - 1),
                                         skip_group_check=True)

                sf = wk.tile([P, HIDDEN], F32, tag="sf", name=f"sf_{lname}_{t}")
                si = wk.tile([P, HIDDEN], F32, tag="si", name=f"si_{lname}_{t}")
                tg = wk.tile([P, HIDDEN], F32, tag="tg", name=f"tg_{lname}_{t}")
                so = wk.tile([P, HIDDEN], F32, tag="so", name=f"so_{lname}_{t}")
                nc.scalar.activation(sf[:], banks[1][:], AF.Sigmoid)
                nc.scalar.activation(si[:], banks[0][:], AF.Sigmoid)
                nc.scalar.activation(tg[:], banks[2][:], AF.Tanh)
                nc.scalar.activation(so[:], banks[3][:], AF.Sigmoid)

                cn = sp.tile([P, HIDDEN], F32, tag=f"c_{lname}", name=f"c_{lname}_{t}")
                nc.vector.tensor_mul(out=sf[:], in0=sf[:], in1=c[:])
                nc.vector.tensor_mul(out=si[:], in0=si[:], in1=tg[:])
                nc.vector.tensor_add(out=cn[:], in0=sf[:], in1=si[:])
                nc.scalar.activation(tg[:], cn[:], AF.Tanh)
                hb = wk.tile([P, HIDDEN], BF16, tag="hb", name=f"hb_{lname}_{t}")
                nc.vector.tensor_mul(out=hb[:], in0=so[:], in1=tg[:])

                ptr = pp.tile([P, KH, P], BF16, tag="tr", name=f"htr_{lname}_{t}")
                for k in range(KH):
                    nc.tensor.transpose(ptr[:, k], hb[:, k * P:(k + 1) * P], ident[:])
                hTn = sp.tile([P, KH, B_LOCAL], BF16, tag=f"hT_{lname}",
                              name=f"hT_{lname}_{t}")
                nc.vector.tensor_copy(out=hTn[:], in_=ptr[:])
                return hTn, cn

            zT0, h0T0, h1T0, c00, c10 = zT, h0T, h1T, c0, c1

            def time_loop():
                ybuf = None
                zT, h0T, h1T, c0, c1 = zT0, h0T0, h1T0, c00, c10
                for t in range(T):
                    h0Tn, c0n = lstm_layer(t, "l0", zT, KX0, h0T, c0, w0x, w0h, b0b)
                    h1Tn, c1n = lstm_layer(t, "l1", h0Tn, KH, h1T, c1, w1x, w1h, b1b)

                    # fc: z' = h1' @ fc_W.T + fc_b, emitted feature-major
                    # [z_feat, B]: stationary fc_W.T chunks, moving h1T; bias
                    # fused per-partition into the ScalarE copyback.
                    zTn = sp.tile([P, KX0, B_LOCAL], BF16, tag="zT", name=f"zT_{t}")
                    pzt = pp.tile([P, KX0, P], F32, tag="tr", name=f"fcT_{t}")
                    for k2 in range(KX0):
                        for k in range(KH):
                            nc.tensor.matmul(pzt[:, k2],
                                             wfc[:, k, k2 * P:(k2 + 1) * P],
                                             h1Tn[:, k],
                                             start=(k == 0), stop=(k == KH - 1))
                    for k2 in range(KX0):
                        nc.scalar.activation(zTn[:, k2], pzt[:, k2], AF.Identity,
                                             bias=bfcT[:, k2:k2 + 1])

                    # lin: out[t] = z' @ lin_W.T + lin_b; bias + copyback fused
                    # into one VectorE add from PSUM.
                    plin = pp.tile([P, OUTPUT], F32, tag="tr", name=f"lin_{t}")
                    for k in range(KX0):
                        nc.tensor.matmul(plin[:], zTn[:, k], wlin[:, k],
                                         start=(k == 0), stop=(k == KX0 - 1))
                    if t % YB == 0:
                        ybuf = wk.tile([P, YB, OUTPUT], F32, tag="ybuf",
                                       name=f"ybuf_{t}")
                    nc.vector.tensor_add(out=ybuf[:, t % YB], in0=plin[:],
                                         in1=blinb[:])
                    if t % YB == YB - 1 or t == T - 1:
                        n = t % YB + 1
                        nc.gpsimd.dma_start(
                            y_ap[t - n + 1:t + 1].rearrange("t b f -> b t f"),
                            ybuf[:, :n])

                    zT, h0T, h1T, c0, c1 = zTn, h0Tn, h1Tn, c0n, c1n

            if rep == 1:
                time_loop()
            else:
                with tc.For_i(0, rep, 1):
                    time_loop()

    nc.compile()
    return nc


def make_in_maps(z0, h0, c0, W_ih0, W_hh0, b_ih0, b_hh0,
                 W_ih1, W_hh1, b_ih1, b_hh1, fc_W, fc_b, lin_W, lin_b):
    f = np.float32
    bf = ml_dtypes.bfloat16
    shared = {
        "w0x": np.ascontiguousarray(W_ih0.T, dtype=bf),
        "w0h": np.ascontiguousarray(W_hh0.T, dtype=bf),
        "w1x": np.ascontiguousarray(W_ih1.T, dtype=bf),
        "w1h": np.ascontiguousarray(W_hh1.T, dtype=bf),
        "wfc": np.ascontiguousarray(fc_W.T, dtype=bf),
        "wlin": np.ascontiguousarray(lin_W.T, dtype=bf),
        "bias0": (b_ih0 + b_hh0).astype(f).reshape(1, H4),
        "bias1": (b_ih1 + b_hh1).astype(f).reshape(1, H4),
        "blin": lin_b.astype(f).reshape(1, OUTPUT),
        "bfcT": np.ascontiguousarray(fc_b.astype(f).reshape(KX0, P).T),
        "ones": np.ones((1, B_LOCAL), dtype=f),
    }
    in_maps = []
    for cidx in range(N_CORES):
        sl = slice(cidx * B_LOCAL, (cidx + 1) * B_LOCAL)
        in_maps.append({
            "zT0": np.ascontiguousarray(z0[sl].T, dtype=bf),
            "h0T_l0": np.ascontiguousarray(h0[0, sl].T, dtype=bf),
            "h0T_l1": np.ascontiguousarray(h0[1, sl].T, dtype=bf),
            "c_l0": np.ascontiguousarray(c0[0, sl], dtype=f),
            "c_l1": np.ascontiguousarray(c0[1, sl], dtype=f),
            **shared,
        })
    return in_maps


_NC_CACHE = {}


def kernel(z0, h0, c0, W_ih0, W_hh0, b_ih0, b_hh0,
           W_ih1, W_hh1, b_ih1, b_hh1, fc_W, fc_b, lin_W, lin_b, T2):
    T = int(T2)
    if T not in _NC_CACHE:
        _NC_CACHE[T] = build(T)
    nc = _NC_CACHE[T]
    in_maps = make_in_maps(z0, h0, c0, W_ih0, W_hh0, b_ih0, b_hh0,
                           W_ih1, W_hh1, b_ih1, b_hh1, fc_W, fc_b, lin_W, lin_b)
    res = run_bass_kernel_spmd(nc, in_maps, list(range(N_CORES)))
    # per-core y: [T, 128, OUTPUT] -> full [T, 1024, OUTPUT]
    return np.concatenate([r["y"] for r in res.results], axis=1)


# revision 20
# speedup vs baseline: 1.3320x; 1.2120x over previous
"""Trainium2 Bass kernel for nn_Decoder (2-layer LSTM decoder, autoregressive).

Reference computation (per timestep t, batch B=1024):
  L0: gates = z @ W_ih0.T + b_ih0 + h0 @ W_hh0.T + b_hh0 ; i,f,g,o = split(gates)
      c0' = sig(f)*c0 + sig(i)*tanh(g) ; h0' = sig(o)*tanh(c0')
  L1: same with h0' as input
  z' = h1' @ fc_W.T + fc_b          (autoregressive feedback)
  out[t] = z' @ lin_W.T + lin_b

Sharding: data-parallel over batch, 8 cores x 128 batch each; weights
replicated and resident in SBUF; the time loop is fully unrolled on-device.

v3 strategy (per core, B=128), all matmul operands bf16 (PSUM f32):
  - Gate matmuls batch-major: PSUM[batch=128, gates=512] per gate bank, with
    the activation (feature-major [feat,128] bf16) stationary and the
    transposed weight [feat, 4H] bf16 moving (1 cycle/row at any N).
  - fc_b is folded algebraically into the L0 gate bias (fc_b @ W_ih0.T, for
    t>=1) and the lin bias (fc_b @ lin_W.T), so the fc bias costs nothing.
    t=0 uses the unfolded gate bias since z0 arrives with no fc applied.
  - Gate bias enters via PSUM preload (ScalarE/VectorE broadcast-copy, off
    the critical path; matmuls accumulate with start=False) or via rank-1
    PE matmuls (bias_mode="pe").
  - PSUM rings: gates rotate through 7 banks so step t+1's banks reuse slots
    freed a full layer earlier; transposes/fc/lin share 1 bank (their uses
    are strictly sequential).
  - sigmoid/tanh on ScalarE out of PSUM into bf16; the whole c-chain runs
    bf16 on VectorE at 16-bit (2x) rate; c-state kept bf16.
  - h' is bf16; PE transposes (bf16 identity, 1 cycle/row) rebuild the
    feature-major operands for the next step.
"""

import sys

sys.path.insert(0, "/opt/trn_rl_repo")

import ml_dtypes
import numpy as np

import concourse.bass as bass
from concourse import bacc, mybir
from concourse.tile import TileContext
from concourse.bass_utils import run_bass_kernel_spmd
from concourse.masks import make_identity

F32 = mybir.dt.float32
F32R = mybir.dt.float32r
BF16 = mybir.dt.bfloat16
AF = mybir.ActivationFunctionType

INPUT, HIDDEN, OUTPUT = 256, 512, 256
H4 = 4 * HIDDEN
B_LOCAL = 128
N_CORES = 8
P = 128
KX0 = INPUT // P   # 2  z feature chunks
KH = HIDDEN // P   # 4  h feature chunks
GATE_ORDER = (1, 0, 2, 3)  # emit f first (the c-chain needs sig(f) first), then i, g, o

YB = 4  # output steps batched per DMA


PRELOAD_PRIO = 1 << 20  # run preloads only when the engine is otherwise idle


def build(T=128, rep=1, bias_mode="actdve", chunk_copies=False, depri=False,
          dma_tr=False):
    nc = bacc.Bacc("TRN2", target_bir_lowering=False, debug=False, num_devices=N_CORES)

    zT_p = nc.declare_dram_parameter("zT0", [INPUT, B_LOCAL], BF16, isOutput=False)
    h0T_p = nc.declare_dram_parameter("h0T_l0", [HIDDEN, B_LOCAL], BF16, isOutput=False)
    h1T_p = nc.declare_dram_parameter("h0T_l1", [HIDDEN, B_LOCAL], BF16, isOutput=False)
    c0_p = nc.declare_dram_parameter("c_l0", [B_LOCAL, HIDDEN], BF16, isOutput=False)
    c1_p = nc.declare_dram_parameter("c_l1", [B_LOCAL, HIDDEN], BF16, isOutput=False)
    w0x_p = nc.declare_dram_parameter("w0x", [INPUT, H4], BF16, isOutput=False)
    w0h_p = nc.declare_dram_parameter("w0h", [HIDDEN, H4], BF16, isOutput=False)
    w1x_p = nc.declare_dram_parameter("w1x", [HIDDEN, H4], BF16, isOutput=False)
    w1h_p = nc.declare_dram_parameter("w1h", [HIDDEN, H4], BF16, isOutput=False)
    wfc_p = nc.declare_dram_parameter("wfc", [HIDDEN, INPUT], BF16, isOutput=False)
    wlin_p = nc.declare_dram_parameter("wlin", [INPUT, OUTPUT], BF16, isOutput=False)
    b0_p = nc.declare_dram_parameter("bias0", [1, H4], F32R, isOutput=False)
    b0f_p = nc.declare_dram_parameter("bias0f", [1, H4], F32R, isOutput=False)
    b1_p = nc.declare_dram_parameter("bias1", [1, H4], F32R, isOutput=False)
    blin_p = nc.declare_dram_parameter("blin", [1, OUTPUT], F32R, isOutput=False)
    ones_p = nc.declare_dram_parameter("ones", [1, B_LOCAL], F32R, isOutput=False)
    y_p = nc.declare_dram_parameter("y", [T, B_LOCAL, OUTPUT], F32, isOutput=True)
    y_ap = y_p[:]

    with TileContext(nc) as tc:
        with (
            tc.tile_pool(name="wpool", bufs=1) as wp,
            tc.tile_pool(name="state", bufs=2) as sp,
            tc.tile_pool(name="work", bufs=2) as wk,
            tc.tile_pool(name="gpsum", bufs=7, space="PSUM") as gp,
            tc.tile_pool(name="spsum", bufs=1, space="PSUM") as pp,
        ):
            # ---- one-time loads: weights, biases, identity, initial state ----
            w0x = wp.tile([P, KX0, H4], BF16, tag="w0x")
            w0h = wp.tile([P, KH, H4], BF16, tag="w0h")
            w1x = wp.tile([P, KH, H4], BF16, tag="w1x")
            w1h = wp.tile([P, KH, H4], BF16, tag="w1h")
            wfc = wp.tile([P, KH, INPUT], BF16, tag="wfc")
            wlin = wp.tile([P, KX0, OUTPUT], BF16, tag="wlin")
            nc.sync.dma_start(w0x[:], w0x_p[:].rearrange("(kc p) n -> p kc n", p=P))
            nc.sync.dma_start(w0h[:], w0h_p[:].rearrange("(kc p) n -> p kc n", p=P))
            nc.sync.dma_start(w1x[:], w1x_p[:].rearrange("(kc p) n -> p kc n", p=P))
            nc.sync.dma_start(w1h[:], w1h_p[:].rearrange("(kc p) n -> p kc n", p=P))
            nc.sync.dma_start(wfc[:], wfc_p[:].rearrange("(kc p) n -> p kc n", p=P))
            nc.sync.dma_start(wlin[:], wlin_p[:].rearrange("(kc p) n -> p kc n", p=P))

            # Bias sources. blin is broadcast to [P, OUTPUT] once; gate biases
            # either stay [1, H4] (rank-1 matmuls per step, bias_mode="pe") or
            # are broadcast once to [P, 4, HIDDEN] (PSUM preloads otherwise).
            blinb = wp.tile([P, OUTPUT], F32, tag="blinb")
            ones = wp.tile([1, B_LOCAL], F32R, tag="ones")
            nc.sync.dma_start(ones[:], ones_p[:])
            if bias_mode == "pe":
                b0 = wp.tile([1, H4], F32R, tag="b0")
                b0f = wp.tile([1, H4], F32R, tag="b0f")
                b1 = wp.tile([1, H4], F32R, tag="b1")
                nc.sync.dma_start(b0[:], b0_p[:])
                nc.sync.dma_start(b0f[:], b0f_p[:])
                nc.sync.dma_start(b1[:], b1_p[:])
                b0b = b1b = b0bf = None
            else:
                b0b = wp.tile([P, 4, HIDDEN], F32, tag="b0b")
                b1b = wp.tile([P, 4, HIDDEN], F32, tag="b1b")
                b0bf = wp.tile([P, 4, HIDDEN], F32, tag="b0bf")
                with tc.tile_pool(name="pre", bufs=1) as pre:
                    b0 = pre.tile([1, H4], F32R, tag="b0")
                    b0f = pre.tile([1, H4], F32R, tag="b0f")
                    b1 = pre.tile([1, H4], F32R, tag="b1")
                    for src_p, src_t, bdst in ((b0_p, b0, b0b), (b0f_p, b0f, b0bf),
                                               (b1_p, b1, b1b)):
                        nc.sync.dma_start(src_t[:], src_p[:])
                        for g in range(4):
                            pb = gp.tile([P, HIDDEN], F32, tag="gb",
                                         name=f"pb_{bdst.name}_{g}")
                            nc.tensor.matmul(pb[:], ones[:],
                                             src_t[:, g * HIDDEN:(g + 1) * HIDDEN],
                                             start=True, stop=True)
                            nc.scalar.activation(bdst[:, g], pb[:], AF.Copy)
            with tc.tile_pool(name="pre2", bufs=1) as pre2:
                blin = pre2.tile([1, OUTPUT], F32R, tag="blin")
                nc.sync.dma_start(blin[:], blin_p[:])
                pl = gp.tile([P, OUTPUT], F32, tag="gb", name="pl_bias")
                nc.tensor.matmul(pl[:], ones[:], blin[:], start=True, stop=True)
                nc.scalar.activation(blinb[:], pl[:], AF.Copy)

            ident = wp.tile([P, P], BF16, tag="ident")
            make_identity(nc, ident[:])

            zT = wp.tile([P, KX0, B_LOCAL], BF16, tag="zT_init")
            h0T = wp.tile([P, KH, B_LOCAL], BF16, tag="h0T_init")
            h1T = wp.tile([P, KH, B_LOCAL], BF16, tag="h1T_init")
            c0 = wp.tile([P, HIDDEN], BF16, tag="c0_init")
            c1 = wp.tile([P, HIDDEN], BF16, tag="c1_init")
            nc.sync.dma_start(zT[:], zT_p[:].rearrange("(kc p) b -> p kc b", p=P))
            nc.sync.dma_start(h0T[:], h0T_p[:].rearrange("(kc p) b -> p kc b", p=P))
            nc.sync.dma_start(h1T[:], h1T_p[:].rearrange("(kc p) b -> p kc b", p=P))
            nc.sync.dma_start(c0[:], c0_p[:])
            nc.sync.dma_start(c1[:], c1_p[:])

            def lstm_layer(t, lname, xT, nx, hT, c, wx, wh, bb, b1d):
                """One LSTM layer step. xT: [P, nx, B] bf16 stationary input
                chunks, hT: [P, KH, B] bf16, c: [P, HIDDEN] bf16.
                Returns (hT_new, c_new)."""
                banks = {}
                for j, g in enumerate(GATE_ORDER):
                    ps = gp.tile([P, HIDDEN], F32, tag="gb", name=f"g_{lname}_{t}_{g}")
                    sl = slice(g * HIDDEN, (g + 1) * HIDDEN)
                    first = False
                    if bias_mode == "pe":
                        nc.tensor.matmul(ps[:], ones[:], b1d[:, sl],
                                         start=True, stop=False)
                    elif j % 2 == 0:
                        pi = nc.scalar.activation(ps[:], bb[:, g], AF.Copy)
                    else:
                        pi = nc.vector.tensor_copy(out=ps[:], in_=bb[:, g])
                    if bias_mode != "pe" and depri:
                        pi.ins.bass_priority = PRELOAD_PRIO + 8 * t + j
                    banks[g] = ps
                # recurrent (h) parts first: ready as soon as last step's hT is
                for j, g in enumerate(GATE_ORDER):
                    ps = banks[g]
                    sl = slice(g * HIDDEN, (g + 1) * HIDDEN)
                    for k in range(KH):
                        nc.tensor.matmul(ps[:], hT[:, k], wh[:, k, sl],
                                         start=False, stop=False,
                                         skip_group_check=True)
                # input (x/z) parts second: they wait on the previous stage
                for j, g in enumerate(GATE_ORDER):
                    ps = banks[g]
                    sl = slice(g * HIDDEN, (g + 1) * HIDDEN)
                    for k in range(nx):
                        nc.tensor.matmul(ps[:], xT[:, k], wx[:, k, sl],
                                         start=False, stop=(k == nx - 1),
                                         skip_group_check=True)

                sf = wk.tile([P, HIDDEN], BF16, tag="sf", name=f"sf_{lname}_{t}")
                si = wk.tile([P, HIDDEN], BF16, tag="si", name=f"si_{lname}_{t}")
                tg = wk.tile([P, HIDDEN], BF16, tag="tg", name=f"tg_{lname}_{t}")
                so = wk.tile([P, HIDDEN], BF16, tag="so", name=f"so_{lname}_{t}")
                nc.scalar.activation(sf[:], banks[1][:], AF.Sigmoid)
                nc.scalar.activation(si[:], banks[0][:], AF.Sigmoid)
                nc.scalar.activation(tg[:], banks[2][:], AF.Tanh)
                nc.scalar.activation(so[:], banks[3][:], AF.Sigmoid)

                cn = sp.tile([P, HIDDEN], BF16, tag=f"c_{lname}", name=f"c_{lname}_{t}")
                nc.vector.tensor_mul(out=sf[:], in0=sf[:], in1=c[:])
                nc.vector.tensor_mul(out=si[:], in0=si[:], in1=tg[:])
                nc.vector.tensor_add(out=cn[:], in0=sf[:], in1=si[:])
                nc.scalar.activation(tg[:], cn[:], AF.Tanh)
                hb = wk.tile([P, HIDDEN], BF16, tag="hb", name=f"hb_{lname}_{t}")
                nc.vector.tensor_mul(out=hb[:], in0=so[:], in1=tg[:])

                hTn = sp.tile([P, KH, B_LOCAL], BF16, tag=f"hT_{lname}",
                              name=f"hT_{lname}_{t}")
                if dma_tr:
                    # XBAR DMA transpose straight to SBUF: no PE/DVE involved
                    nc.sync.dma_start_transpose(hTn[:], hb[:])
                    return hTn, cn
                ptr = pp.tile([P, KH, P], BF16, tag="tr", name=f"htr_{lname}_{t}")
                # per-chunk transpose+copy so chunk-k consumers start without
                # waiting for the whole tile
                if chunk_copies:
                    for k in range(KH):
                        nc.tensor.transpose(ptr[:, k], hb[:, k * P:(k + 1) * P],
                                            ident[:])
                        nc.vector.tensor_copy(out=hTn[:, k], in_=ptr[:, k])
                else:
                    for k in range(KH):
                        nc.tensor.transpose(ptr[:, k], hb[:, k * P:(k + 1) * P],
                                            ident[:])
                    nc.vector.tensor_copy(out=hTn[:], in_=ptr[:])
                return hTn, cn

            zT0, h0T0, h1T0, c00, c10 = zT, h0T, h1T, c0, c1

            def time_loop(first_pass):
                ybuf = None
                zT, h0T, h1T, c0, c1 = zT0, h0T0, h1T0, c00, c10
                for t in range(T):
                    if first_pass and t == 0:
                        bb0, b1d0 = b0bf, (b0f if bias_mode == "pe" else None)
                    else:
                        bb0, b1d0 = b0b, (b0 if bias_mode == "pe" else None)
                    h0Tn, c0n = lstm_layer(t, "l0", zT, KX0, h0T, c0, w0x, w0h,
                                           bb0, b1d0)
                    h1Tn, c1n = lstm_layer(t, "l1", h0Tn, KH, h1T, c1, w1x, w1h,
                                           b1b, (b1 if bias_mode == "pe" else None))

                    # fc: z' = h1' @ fc_W.T (bias folded away), feature-major
                    # [z_feat, B]: stationary fc_W.T chunks, moving h1T.
                    zTn = sp.tile([P, KX0, B_LOCAL], BF16, tag="zT", name=f"zT_{t}")
                    pzt = pp.tile([P, KX0, P], F32, tag="tr", name=f"fcT_{t}")
                    for k2 in range(KX0):
                        for k in range(KH):
                            nc.tensor.matmul(pzt[:, k2],
                                             wfc[:, k, k2 * P:(k2 + 1) * P],
                                             h1Tn[:, k],
                                             start=(k == 0), stop=(k == KH - 1))
                    nc.scalar.activation(zTn[:], pzt[:], AF.Copy)

                    # lin: out[t] = z' @ lin_W.T + lin_b'; bias + copyback fused
                    # into one VectorE add from PSUM.
                    plin = pp.tile([P, OUTPUT], F32, tag="tr", name=f"lin_{t}")
                    for k in range(KX0):
                        nc.tensor.matmul(plin[:], zTn[:, k], wlin[:, k],
                                         start=(k == 0), stop=(k == KX0 - 1))
                    if t % YB == 0:
                        ybuf = wk.tile([P, YB, OUTPUT], F32, tag="ybuf",
                                       name=f"ybuf_{t}")
                    yi = nc.vector.tensor_add(out=ybuf[:, t % YB], in0=plin[:],
                                              in1=blinb[:])
                    if depri:
                        yi.ins.bass_priority = PRELOAD_PRIO + 8 * t + 6
                    if t % YB == YB - 1 or t == T - 1:
                        n = t % YB + 1
                        nc.gpsimd.dma_start(
                            y_ap[t - n + 1:t + 1].rearrange("t b f -> b t f"),
                            ybuf[:, :n])

                    zT, h0T, h1T, c0, c1 = zTn, h0Tn, h1Tn, c0n, c1n

            if rep == 1:
                time_loop(True)
            else:
                with tc.For_i(0, rep, 1):
                    time_loop(False)

    nc.compile()
    return nc


def make_in_maps(z0, h0, c0, W_ih0, W_hh0, b_ih0, b_hh0,
                 W_ih1, W_hh1, b_ih1, b_hh1, fc_W, fc_b, lin_W, lin_b):
    f = np.float32
    bf = ml_dtypes.bfloat16
    b0_raw = (b_ih0 + b_hh0).astype(f)
    fold = fc_b.astype(f) @ np.asarray(W_ih0, f).T
    shared = {
        "w0x": np.ascontiguousarray(W_ih0.T, dtype=bf),
        "w0h": np.ascontiguousarray(W_hh0.T, dtype=bf),
        "w1x": np.ascontiguousarray(W_ih1.T, dtype=bf),
        "w1h": np.ascontiguousarray(W_hh1.T, dtype=bf),
        "wfc": np.ascontiguousarray(fc_W.T, dtype=bf),
        "wlin": np.ascontiguousarray(lin_W.T, dtype=bf),
        "bias0": (b0_raw + fold).reshape(1, H4),
        "bias0f": b0_raw.reshape(1, H4),
        "bias1": (b_ih1 + b_hh1).astype(f).reshape(1, H4),
        "blin": (lin_b.astype(f) + fc_b.astype(f) @ np.asarray(lin_W, f).T
                 ).reshape(1, OUTPUT),
        "ones": np.ones((1, B_LOCAL), dtype=f),
    }
    in_maps = []
    for cidx in range(N_CORES):
        sl = slice(cidx * B_LOCAL, (cidx + 1) * B_LOCAL)
        in_maps.append({
            "zT0": np.ascontiguousarray(z0[sl].T, dtype=bf),
            "h0T_l0": np.ascontiguousarray(h0[0, sl].T, dtype=bf),
            "h0T_l1": np.ascontiguousarray(h0[1, sl].T, dtype=bf),
            "c_l0": np.ascontiguousarray(c0[0, sl], dtype=bf),
            "c_l1": np.ascontiguousarray(c0[1, sl], dtype=bf),
            **shared,
        })
    return in_maps


_NC_CACHE = {}


def kernel(z0, h0, c0, W_ih0, W_hh0, b_ih0, b_hh0,
           W_ih1, W_hh1, b_ih1, b_hh1, fc_W, fc_b, lin_W, lin_b, T2):
    T = int(T2)
    if T not in _NC_CACHE:
        _NC_CACHE[T] = build(T)
    nc = _NC_CACHE[T]
    in_maps = make_in_maps(z0, h0, c0, W_ih0, W_hh0, b_ih0, b_hh0,
                           W_ih1, W_hh1, b_ih1, b_hh1, fc_W, fc_b, lin_W, lin_b)
    res = run_bass_kernel_spmd(nc, in_maps, list(range(N_CORES)))
    # per-core y: [T, 128, OUTPUT] -> full [T, 1024, OUTPUT]
    return np.concatenate([r["y"] for r in res.results], axis=1)


# revision 24
# speedup vs baseline: 1.3367x; 1.0036x over previous
"""Trainium2 Bass kernel for nn_Decoder (2-layer LSTM decoder, autoregressive).

Reference computation (per timestep t, batch B=1024):
  L0: gates = z @ W_ih0.T + b_ih0 + h0 @ W_hh0.T + b_hh0 ; i,f,g,o = split(gates)
      c0' = sig(f)*c0 + sig(i)*tanh(g) ; h0' = sig(o)*tanh(c0')
  L1: same with h0' as input
  z' = h1' @ fc_W.T + fc_b          (autoregressive feedback)
  out[t] = z' @ lin_W.T + lin_b

Sharding: data-parallel over batch, 8 cores x 128 batch each; weights
replicated and resident in SBUF; the time loop is fully unrolled on-device.

v3 strategy (per core, B=128), all matmul operands bf16 (PSUM f32):
  - Gate matmuls batch-major: PSUM[batch=128, gates=512] per gate bank, with
    the activation (feature-major [feat,128] bf16) stationary and the
    transposed weight [feat, 4H] bf16 moving (1 cycle/row at any N).
  - fc_b is folded algebraically into the L0 gate bias (fc_b @ W_ih0.T, for
    t>=1) and the lin bias (fc_b @ lin_W.T), so the fc bias costs nothing.
    t=0 uses the unfolded gate bias since z0 arrives with no fc applied.
  - Gate bias enters via PSUM preload (ScalarE/VectorE broadcast-copy, off
    the critical path; matmuls accumulate with start=False) or via rank-1
    PE matmuls (bias_mode="pe").
  - PSUM rings: gates rotate through 7 banks so step t+1's banks reuse slots
    freed a full layer earlier; transposes/fc/lin share 1 bank (their uses
    are strictly sequential).
  - sigmoid/tanh on ScalarE out of PSUM into bf16; the whole c-chain runs
    bf16 on VectorE at 16-bit (2x) rate; c-state kept bf16.
  - h' is bf16; PE transposes (bf16 identity, 1 cycle/row) rebuild the
    feature-major operands for the next step.
"""

import sys

sys.path.insert(0, "/opt/trn_rl_repo")

import ml_dtypes
import numpy as np

import concourse.bass as bass
from concourse import bacc, mybir
from concourse.tile import TileContext
from concourse.bass_utils import run_bass_kernel_spmd
from concourse.masks import make_identity

F32 = mybir.dt.float32
F32R = mybir.dt.float32r
BF16 = mybir.dt.bfloat16
AF = mybir.ActivationFunctionType

INPUT, HIDDEN, OUTPUT = 256, 512, 256
H4 = 4 * HIDDEN
B_LOCAL = 128
N_CORES = 8
P = 128
KX0 = INPUT // P   # 2  z feature chunks
KH = HIDDEN // P   # 4  h feature chunks
GATE_ORDER = (1, 0, 2, 3)  # emit f first (the c-chain needs sig(f) first), then i, g, o

YB = 4  # output steps batched per DMA


PRELOAD_PRIO = 1 << 20  # run preloads only when the engine is otherwise idle


def build(T=128, rep=1, bias_mode="actdve", chunk_copies=False, depri=False,
          dma_tr=False, act_f32=False, kmajor=False):
    nc = bacc.Bacc("TRN2", target_bir_lowering=False, debug=False, num_devices=N_CORES)

    zT_p = nc.declare_dram_parameter("zT0", [INPUT, B_LOCAL], BF16, isOutput=False)
    h0T_p = nc.declare_dram_parameter("h0T_l0", [HIDDEN, B_LOCAL], BF16, isOutput=False)
    h1T_p = nc.declare_dram_parameter("h0T_l1", [HIDDEN, B_LOCAL], BF16, isOutput=False)
    c0_p = nc.declare_dram_parameter("c_l0", [B_LOCAL, HIDDEN], BF16, isOutput=False)
    c1_p = nc.declare_dram_parameter("c_l1", [B_LOCAL, HIDDEN], BF16, isOutput=False)
    w0x_p = nc.declare_dram_parameter("w0x", [INPUT, H4], BF16, isOutput=False)
    w0h_p = nc.declare_dram_parameter("w0h", [HIDDEN, H4], BF16, isOutput=False)
    w1x_p = nc.declare_dram_parameter("w1x", [HIDDEN, H4], BF16, isOutput=False)
    w1h_p = nc.declare_dram_parameter("w1h", [HIDDEN, H4], BF16, isOutput=False)
    wfc_p = nc.declare_dram_parameter("wfc", [HIDDEN, INPUT], BF16, isOutput=False)
    wlin_p = nc.declare_dram_parameter("wlin", [INPUT, OUTPUT], BF16, isOutput=False)
    b0_p = nc.declare_dram_parameter("bias0", [1, H4], F32R, isOutput=False)
    b0f_p = nc.declare_dram_parameter("bias0f", [1, H4], F32R, isOutput=False)
    b1_p = nc.declare_dram_parameter("bias1", [1, H4], F32R, isOutput=False)
    blin_p = nc.declare_dram_parameter("blin", [1, OUTPUT], F32R, isOutput=False)
    ones_p = nc.declare_dram_parameter("ones", [1, B_LOCAL], F32R, isOutput=False)
    y_p = nc.declare_dram_parameter("y", [T, B_LOCAL, OUTPUT], F32, isOutput=True)
    y_ap = y_p[:]

    with TileContext(nc) as tc:
        with (
            tc.tile_pool(name="wpool", bufs=1) as wp,
            tc.tile_pool(name="state", bufs=2) as sp,
            tc.tile_pool(name="work", bufs=2) as wk,
            tc.tile_pool(name="gpsum", bufs=7, space="PSUM") as gp,
            tc.tile_pool(name="spsum", bufs=1, space="PSUM") as pp,
        ):
            # ---- one-time loads: weights, biases, identity, initial state ----
            w0x = wp.tile([P, KX0, H4], BF16, tag="w0x")
            w0h = wp.tile([P, KH, H4], BF16, tag="w0h")
            w1x = wp.tile([P, KH, H4], BF16, tag="w1x")
            w1h = wp.tile([P, KH, H4], BF16, tag="w1h")
            wfc = wp.tile([P, KH, INPUT], BF16, tag="wfc")
            wlin = wp.tile([P, KX0, OUTPUT], BF16, tag="wlin")
            nc.sync.dma_start(w0x[:], w0x_p[:].rearrange("(kc p) n -> p kc n", p=P))
            nc.sync.dma_start(w0h[:], w0h_p[:].rearrange("(kc p) n -> p kc n", p=P))
            nc.sync.dma_start(w1x[:], w1x_p[:].rearrange("(kc p) n -> p kc n", p=P))
            nc.sync.dma_start(w1h[:], w1h_p[:].rearrange("(kc p) n -> p kc n", p=P))
            nc.sync.dma_start(wfc[:], wfc_p[:].rearrange("(kc p) n -> p kc n", p=P))
            nc.sync.dma_start(wlin[:], wlin_p[:].rearrange("(kc p) n -> p kc n", p=P))

            # Bias sources. blin is broadcast to [P, OUTPUT] once; gate biases
            # either stay [1, H4] (rank-1 matmuls per step, bias_mode="pe") or
            # are broadcast once to [P, 4, HIDDEN] (PSUM preloads otherwise).
            blinb = wp.tile([P, OUTPUT], F32, tag="blinb")
            ones = wp.tile([1, B_LOCAL], F32R, tag="ones")
            nc.sync.dma_start(ones[:], ones_p[:])
            if bias_mode == "pe":
                b0 = wp.tile([1, H4], F32R, tag="b0")
                b0f = wp.tile([1, H4], F32R, tag="b0f")
                b1 = wp.tile([1, H4], F32R, tag="b1")
                nc.sync.dma_start(b0[:], b0_p[:])
                nc.sync.dma_start(b0f[:], b0f_p[:])
                nc.sync.dma_start(b1[:], b1_p[:])
                b0b = b1b = b0bf = None
            else:
                b0b = wp.tile([P, 4, HIDDEN], F32, tag="b0b")
                b1b = wp.tile([P, 4, HIDDEN], F32, tag="b1b")
                b0bf = wp.tile([P, 4, HIDDEN], F32, tag="b0bf")
                with tc.tile_pool(name="pre", bufs=1) as pre:
                    b0 = pre.tile([1, H4], F32R, tag="b0")
                    b0f = pre.tile([1, H4], F32R, tag="b0f")
                    b1 = pre.tile([1, H4], F32R, tag="b1")
                    for src_p, src_t, bdst in ((b0_p, b0, b0b), (b0f_p, b0f, b0bf),
                                               (b1_p, b1, b1b)):
                        nc.sync.dma_start(src_t[:], src_p[:])
                        for g in range(4):
                            pb = gp.tile([P, HIDDEN], F32, tag="gb",
                                         name=f"pb_{bdst.name}_{g}")
                            nc.tensor.matmul(pb[:], ones[:],
                                             src_t[:, g * HIDDEN:(g + 1) * HIDDEN],
                                             start=True, stop=True)
                            nc.scalar.activation(bdst[:, g], pb[:], AF.Copy)
            with tc.tile_pool(name="pre2", bufs=1) as pre2:
                blin = pre2.tile([1, OUTPUT], F32R, tag="blin")
                nc.sync.dma_start(blin[:], blin_p[:])
                pl = gp.tile([P, OUTPUT], F32, tag="gb", name="pl_bias")
                nc.tensor.matmul(pl[:], ones[:], blin[:], start=True, stop=True)
                nc.scalar.activation(blinb[:], pl[:], AF.Copy)

            ident = wp.tile([P, P], BF16, tag="ident")
            make_identity(nc, ident[:])

            zT = wp.tile([P, KX0, B_LOCAL], BF16, tag="zT_init")
            h0T = wp.tile([P, KH, B_LOCAL], BF16, tag="h0T_init")
            h1T = wp.tile([P, KH, B_LOCAL], BF16, tag="h1T_init")
            c0 = wp.tile([P, HIDDEN], BF16, tag="c0_init")
            c1 = wp.tile([P, HIDDEN], BF16, tag="c1_init")
            nc.sync.dma_start(zT[:], zT_p[:].rearrange("(kc p) b -> p kc b", p=P))
            nc.sync.dma_start(h0T[:], h0T_p[:].rearrange("(kc p) b -> p kc b", p=P))
            nc.sync.dma_start(h1T[:], h1T_p[:].rearrange("(kc p) b -> p kc b", p=P))
            nc.sync.dma_start(c0[:], c0_p[:])
            nc.sync.dma_start(c1[:], c1_p[:])

            def lstm_layer(t, lname, xT, nx, hT, c, wx, wh, bb, b1d):
                """One LSTM layer step. xT: [P, nx, B] bf16 stationary input
                chunks, hT: [P, KH, B] bf16, c: [P, HIDDEN] bf16.
                Returns (hT_new, c_new)."""
                banks = {}
                for j, g in enumerate(GATE_ORDER):
                    ps = gp.tile([P, HIDDEN], F32, tag="gb", name=f"g_{lname}_{t}_{g}")
                    sl = slice(g * HIDDEN, (g + 1) * HIDDEN)
                    first = False
                    if bias_mode == "pe":
                        nc.tensor.matmul(ps[:], ones[:], b1d[:, sl],
                                         start=True, stop=False)
                    elif j % 2 == 0:
                        pi = nc.scalar.activation(ps[:], bb[:, g], AF.Copy)
                    else:
                        pi = nc.vector.tensor_copy(out=ps[:], in_=bb[:, g])
                    if bias_mode != "pe" and depri:
                        pi.ins.bass_priority = PRELOAD_PRIO + 8 * t + j
                    banks[g] = ps
                # recurrent (h) parts first: ready as soon as last step's hT is
                if kmajor:
                    # k-major: 4 consecutive matmuls share one stationary chunk
                    for k in range(KH):
                        for j, g in enumerate(GATE_ORDER):
                            sl = slice(g * HIDDEN, (g + 1) * HIDDEN)
                            nc.tensor.matmul(banks[g][:], hT[:, k], wh[:, k, sl],
                                             start=False, stop=False,
                                             skip_group_check=True)
                    for k in range(nx):
                        for j, g in enumerate(GATE_ORDER):
                            sl = slice(g * HIDDEN, (g + 1) * HIDDEN)
                            nc.tensor.matmul(banks[g][:], xT[:, k], wx[:, k, sl],
                                             start=False, stop=(k == nx - 1),
                                             skip_group_check=True)
                else:
                    for j, g in enumerate(GATE_ORDER):
                        ps = banks[g]
                        sl = slice(g * HIDDEN, (g + 1) * HIDDEN)
                        for k in range(KH):
                            nc.tensor.matmul(ps[:], hT[:, k], wh[:, k, sl],
                                             start=False, stop=False,
                                             skip_group_check=True)
                    # input (x/z) parts second: they wait on the previous stage
                    for j, g in enumerate(GATE_ORDER):
                        ps = banks[g]
                        sl = slice(g * HIDDEN, (g + 1) * HIDDEN)
                        for k in range(nx):
                            nc.tensor.matmul(ps[:], xT[:, k], wx[:, k, sl],
                                             start=False, stop=(k == nx - 1),
                                             skip_group_check=True)

                AD = F32 if act_f32 else BF16
                sf = wk.tile([P, HIDDEN], AD, tag="sf", name=f"sf_{lname}_{t}")
                si = wk.tile([P, HIDDEN], AD, tag="si", name=f"si_{lname}_{t}")
                tg = wk.tile([P, HIDDEN], AD, tag="tg", name=f"tg_{lname}_{t}")
                so = wk.tile([P, HIDDEN], AD, tag="so", name=f"so_{lname}_{t}")
                nc.scalar.activation(sf[:], banks[1][:], AF.Sigmoid)
                nc.scalar.activation(si[:], banks[0][:], AF.Sigmoid)
                nc.scalar.activation(tg[:], banks[2][:], AF.Tanh)
                nc.scalar.activation(so[:], banks[3][:], AF.Sigmoid)

                cn = sp.tile([P, HIDDEN], BF16, tag=f"c_{lname}", name=f"c_{lname}_{t}")
                nc.vector.tensor_mul(out=sf[:], in0=sf[:], in1=c[:])
                nc.vector.tensor_mul(out=si[:], in0=si[:], in1=tg[:])
                nc.vector.tensor_add(out=cn[:], in0=sf[:], in1=si[:])
                nc.scalar.activation(tg[:], cn[:], AF.Tanh)
                hb = wk.tile([P, HIDDEN], BF16, tag="hb", name=f"hb_{lname}_{t}")
                nc.vector.tensor_mul(out=hb[:], in0=so[:], in1=tg[:])

                hTn = sp.tile([P, KH, B_LOCAL], BF16, tag=f"hT_{lname}",
                              name=f"hT_{lname}_{t}")
                if dma_tr:
                    # XBAR DMA transpose straight to SBUF: no PE/DVE involved
                    nc.sync.dma_start_transpose(hTn[:], hb[:])
                    return hTn, cn
                ptr = pp.tile([P, KH, P], BF16, tag="tr", name=f"htr_{lname}_{t}")
                # per-chunk transpose+copy so chunk-k consumers start without
                # waiting for the whole tile
                if chunk_copies:
                    for k in range(KH):
                        nc.tensor.transpose(ptr[:, k], hb[:, k * P:(k + 1) * P],
                                            ident[:])
                        nc.vector.tensor_copy(out=hTn[:, k], in_=ptr[:, k])
                else:
                    for k in range(KH):
                        nc.tensor.transpose(ptr[:, k], hb[:, k * P:(k + 1) * P],
                                            ident[:])
                    nc.vector.tensor_copy(out=hTn[:], in_=ptr[:])
                return hTn, cn

            zT0, h0T0, h1T0, c00, c10 = zT, h0T, h1T, c0, c1

            def time_loop(first_pass):
                ybuf = None
                zT, h0T, h1T, c0, c1 = zT0, h0T0, h1T0, c00, c10
                for t in range(T):
                    if first_pass and t == 0:
                        bb0, b1d0 = b0bf, (b0f if bias_mode == "pe" else None)
                    else:
                        bb0, b1d0 = b0b, (b0 if bias_mode == "pe" else None)
                    h0Tn, c0n = lstm_layer(t, "l0", zT, KX0, h0T, c0, w0x, w0h,
                                           bb0, b1d0)
                    h1Tn, c1n = lstm_layer(t, "l1", h0Tn, KH, h1T, c1, w1x, w1h,
                                           b1b, (b1 if bias_mode == "pe" else None))

                    # fc: z' = h1' @ fc_W.T (bias folded away), feature-major
                    # [z_feat, B]: stationary fc_W.T chunks, moving h1T.
                    zTn = sp.tile([P, KX0, B_LOCAL], BF16, tag="zT", name=f"zT_{t}")
                    pzt = pp.tile([P, KX0, P], F32, tag="tr", name=f"fcT_{t}")
                    for k2 in range(KX0):
                        for k in range(KH):
                            nc.tensor.matmul(pzt[:, k2],
                                             wfc[:, k, k2 * P:(k2 + 1) * P],
                                             h1Tn[:, k],
                                             start=(k == 0), stop=(k == KH - 1))
                    nc.scalar.activation(zTn[:], pzt[:], AF.Copy)

                    # lin: out[t] = z' @ lin_W.T + lin_b'; bias + copyback fused
                    # into one VectorE add from PSUM.
                    plin = pp.tile([P, OUTPUT], F32, tag="tr", name=f"lin_{t}")
                    for k in range(KX0):
                        nc.tensor.matmul(plin[:], zTn[:, k], wlin[:, k],
                                         start=(k == 0), stop=(k == KX0 - 1))
                    if t % YB == 0:
                        ybuf = wk.tile([P, YB, OUTPUT], F32, tag="ybuf",
                                       name=f"ybuf_{t}")
                    yi = nc.vector.tensor_add(out=ybuf[:, t % YB], in0=plin[:],
                                              in1=blinb[:])
                    if depri:
                        yi.ins.bass_priority = PRELOAD_PRIO + 8 * t + 6
                    if t % YB == YB - 1 or t == T - 1:
                        n = t % YB + 1
                        nc.gpsimd.dma_start(
                            y_ap[t - n + 1:t + 1].rearrange("t b f -> b t f"),
                            ybuf[:, :n])

                    zT, h0T, h1T, c0, c1 = zTn, h0Tn, h1Tn, c0n, c1n

            if rep == 1:
                time_loop(True)
            else:
                with tc.For_i(0, rep, 1):
                    time_loop(False)

    nc.compile()
    return nc


def make_in_maps(z0, h0, c0, W_ih0, W_hh0, b_ih0, b_hh0,
                 W_ih1, W_hh1, b_ih1, b_hh1, fc_W, fc_b, lin_W, lin_b):
    f = np.float32
    bf = ml_dtypes.bfloat16
    b0_raw = (b_ih0 + b_hh0).astype(f)
    fold = fc_b.astype(f) @ np.asarray(W_ih0, f).T
    shared = {
        "w0x": np.ascontiguousarray(W_ih0.T, dtype=bf),
        "w0h": np.ascontiguousarray(W_hh0.T, dtype=bf),
        "w1x": np.ascontiguousarray(W_ih1.T, dtype=bf),
        "w1h": np.ascontiguousarray(W_hh1.T, dtype=bf),
        "wfc": np.ascontiguousarray(fc_W.T, dtype=bf),
        "wlin": np.ascontiguousarray(lin_W.T, dtype=bf),
        "bias0": (b0_raw + fold).reshape(1, H4),
        "bias0f": b0_raw.reshape(1, H4),
        "bias1": (b_ih1 + b_hh1).astype(f).reshape(1, H4),
        "blin": (lin_b.astype(f) + fc_b.astype(f) @ np.asarray(lin_W, f).T
                 ).reshape(1, OUTPUT),
        "ones": np.ones((1, B_LOCAL), dtype=f),
    }
    in_maps = []
    for cidx in range(N_CORES):
        sl = slice(cidx * B_LOCAL, (cidx + 1) * B_LOCAL)
        in_maps.append({
            "zT0": np.ascontiguousarray(z0[sl].T, dtype=bf),
            "h0T_l0": np.ascontiguousarray(h0[0, sl].T, dtype=bf),
            "h0T_l1": np.ascontiguousarray(h0[1, sl].T, dtype=bf),
            "c_l0": np.ascontiguousarray(c0[0, sl], dtype=bf),
            "c_l1": np.ascontiguousarray(c0[1, sl], dtype=bf),
            **shared,
        })
    return in_maps


_NC_CACHE = {}


def kernel(z0, h0, c0, W_ih0, W_hh0, b_ih0, b_hh0,
           W_ih1, W_hh1, b_ih1, b_hh1, fc_W, fc_b, lin_W, lin_b, T2):
    T = int(T2)
    if T not in _NC_CACHE:
        _NC_CACHE[T] = build(T)
    nc = _NC_CACHE[T]
    in_maps = make_in_maps(z0, h0, c0, W_ih0, W_hh0, b_ih0, b_hh0,
                           W_ih1, W_hh1, b_ih1, b_hh1, fc_W, fc_b, lin_W, lin_b)
    res = run_bass_kernel_spmd(nc, in_maps, list(range(N_CORES)))
    # per-core y: [T, 128, OUTPUT] -> full [T, 1024, OUTPUT]
    return np.concatenate([r["y"] for r in res.results], axis=1)


# revision 30
# speedup vs baseline: 1.3676x; 1.0231x over previous
"""Trainium2 Bass kernel for nn_Decoder (2-layer LSTM decoder, autoregressive).

Reference computation (per timestep t, batch B=1024):
  L0: gates = z @ W_ih0.T + b_ih0 + h0 @ W_hh0.T + b_hh0 ; i,f,g,o = split(gates)
      c0' = sig(f)*c0 + sig(i)*tanh(g) ; h0' = sig(o)*tanh(c0')
  L1: same with h0' as input
  z' = h1' @ fc_W.T + fc_b          (autoregressive feedback)
  out[t] = z' @ lin_W.T + lin_b

Sharding: data-parallel over batch, 8 cores x 128 batch each; weights
replicated and resident in SBUF; the time loop is fully unrolled on-device.

v3 strategy (per core, B=128), all matmul operands bf16 (PSUM f32):
  - Gate matmuls batch-major: PSUM[batch=128, gates=512] per gate bank, with
    the activation (feature-major [feat,128] bf16) stationary and the
    transposed weight [feat, 4H] bf16 moving (1 cycle/row at any N).
  - fc_b is folded algebraically into the L0 gate bias (fc_b @ W_ih0.T, for
    t>=1) and the lin bias (fc_b @ lin_W.T), so the fc bias costs nothing.
    t=0 uses the unfolded gate bias since z0 arrives with no fc applied.
  - Gate bias enters via PSUM preload (ScalarE/VectorE broadcast-copy, off
    the critical path; matmuls accumulate with start=False) or via rank-1
    PE matmuls (bias_mode="pe").
  - PSUM rings: gates rotate through 7 banks so step t+1's banks reuse slots
    freed a full layer earlier; transposes/fc/lin share 1 bank (their uses
    are strictly sequential).
  - sigmoid/tanh on ScalarE out of PSUM into bf16; the whole c-chain runs
    bf16 on VectorE at 16-bit (2x) rate; c-state kept bf16.
  - h' is bf16; PE transposes (bf16 identity, 1 cycle/row) rebuild the
    feature-major operands for the next step.
"""

import sys

sys.path.insert(0, "/opt/trn_rl_repo")

import ml_dtypes
import numpy as np

import concourse.bass as bass
from concourse import bacc, mybir
from concourse.tile import TileContext
from concourse.bass_utils import run_bass_kernel_spmd
from concourse.masks import make_identity

F32 = mybir.dt.float32
F32R = mybir.dt.float32r
BF16 = mybir.dt.bfloat16
AF = mybir.ActivationFunctionType

INPUT, HIDDEN, OUTPUT = 256, 512, 256
H4 = 4 * HIDDEN
B_LOCAL = 128
N_CORES = 8
P = 128
KX0 = INPUT // P   # 2  z feature chunks
KH = HIDDEN // P   # 4  h feature chunks
GATE_ORDER = (1, 0, 2, 3)  # emit f first (the c-chain needs sig(f) first), then i, g, o

YB = 4  # output steps batched per DMA


PRELOAD_PRIO = 1 << 20  # run preloads only when the engine is otherwise idle


def build(T=128, rep=1, bias_mode="actdve", chunk_copies=False, depri=False,
          dma_tr=False, act_f32=False, kmajor=False):
    nc = bacc.Bacc("TRN2", target_bir_lowering=False, debug=False, num_devices=N_CORES)

    zT_p = nc.declare_dram_parameter("zT0", [INPUT, B_LOCAL], BF16, isOutput=False)
    h0T_p = nc.declare_dram_parameter("h0T_l0", [HIDDEN, B_LOCAL], BF16, isOutput=False)
    h1T_p = nc.declare_dram_parameter("h0T_l1", [HIDDEN, B_LOCAL], BF16, isOutput=False)
    c0_p = nc.declare_dram_parameter("c_l0", [B_LOCAL, HIDDEN], BF16, isOutput=False)
    c1_p = nc.declare_dram_parameter("c_l1", [B_LOCAL, HIDDEN], BF16, isOutput=False)
    w0x_p = nc.declare_dram_parameter("w0x", [INPUT, H4], BF16, isOutput=False)
    w0h_p = nc.declare_dram_parameter("w0h", [HIDDEN, H4], BF16, isOutput=False)
    w1x_p = nc.declare_dram_parameter("w1x", [HIDDEN, H4], BF16, isOutput=False)
    w1h_p = nc.declare_dram_parameter("w1h", [HIDDEN, H4], BF16, isOutput=False)
    wfc_p = nc.declare_dram_parameter("wfc", [HIDDEN, INPUT], BF16, isOutput=False)
    wzy_p = nc.declare_dram_parameter("wzy", [HIDDEN, OUTPUT], BF16, isOutput=False)
    b0_p = nc.declare_dram_parameter("bias0", [1, H4], F32R, isOutput=False)
    b0f_p = nc.declare_dram_parameter("bias0f", [1, H4], F32R, isOutput=False)
    b1_p = nc.declare_dram_parameter("bias1", [1, H4], F32R, isOutput=False)
    blin_p = nc.declare_dram_parameter("blin", [1, OUTPUT], F32R, isOutput=False)
    ones_p = nc.declare_dram_parameter("ones", [1, B_LOCAL], F32R, isOutput=False)
    y_p = nc.declare_dram_parameter("y", [T, B_LOCAL, OUTPUT], F32, isOutput=True)
    y_ap = y_p[:]

    with TileContext(nc) as tc:
        with (
            tc.tile_pool(name="wpool", bufs=1) as wp,
            tc.tile_pool(name="state", bufs=2) as sp,
            tc.tile_pool(name="work", bufs=2) as wk,
            tc.tile_pool(name="gpsum", bufs=7, space="PSUM") as gp,
            tc.tile_pool(name="spsum", bufs=1, space="PSUM") as pp,
        ):
            # ---- one-time loads: weights, biases, identity, initial state ----
            w0x = wp.tile([P, KX0, H4], BF16, tag="w0x")
            w0h = wp.tile([P, KH, H4], BF16, tag="w0h")
            w1x = wp.tile([P, KH, H4], BF16, tag="w1x")
            w1h = wp.tile([P, KH, H4], BF16, tag="w1h")
            wfc = wp.tile([P, KH, INPUT], BF16, tag="wfc")
            wzy = wp.tile([P, KH, OUTPUT], BF16, tag="wzy")
            nc.sync.dma_start(w0x[:], w0x_p[:].rearrange("(kc p) n -> p kc n", p=P))
            nc.sync.dma_start(w0h[:], w0h_p[:].rearrange("(kc p) n -> p kc n", p=P))
            nc.sync.dma_start(w1x[:], w1x_p[:].rearrange("(kc p) n -> p kc n", p=P))
            nc.sync.dma_start(w1h[:], w1h_p[:].rearrange("(kc p) n -> p kc n", p=P))
            nc.sync.dma_start(wfc[:], wfc_p[:].rearrange("(kc p) n -> p kc n", p=P))
            nc.sync.dma_start(wzy[:], wzy_p[:].rearrange("(kc p) n -> p kc n", p=P))

            # Bias sources. blin is broadcast to [P, OUTPUT] once; gate biases
            # either stay [1, H4] (rank-1 matmuls per step, bias_mode="pe") or
            # are broadcast once to [P, 4, HIDDEN] (PSUM preloads otherwise).
            blinb = wp.tile([P, OUTPUT], F32, tag="blinb")
            ones = wp.tile([1, B_LOCAL], F32R, tag="ones")
            nc.sync.dma_start(ones[:], ones_p[:])
            if bias_mode == "pe":
                b0 = wp.tile([1, H4], F32R, tag="b0")
                b0f = wp.tile([1, H4], F32R, tag="b0f")
                b1 = wp.tile([1, H4], F32R, tag="b1")
                nc.sync.dma_start(b0[:], b0_p[:])
                nc.sync.dma_start(b0f[:], b0f_p[:])
                nc.sync.dma_start(b1[:], b1_p[:])
                b0b = b1b = b0bf = None
            else:
                b0b = wp.tile([P, 4, HIDDEN], F32, tag="b0b")
                b1b = wp.tile([P, 4, HIDDEN], F32, tag="b1b")
                b0bf = wp.tile([P, 4, HIDDEN], F32, tag="b0bf")
                with tc.tile_pool(name="pre", bufs=1) as pre:
                    b0 = pre.tile([1, H4], F32R, tag="b0")
                    b0f = pre.tile([1, H4], F32R, tag="b0f")
                    b1 = pre.tile([1, H4], F32R, tag="b1")
                    for src_p, src_t, bdst in ((b0_p, b0, b0b), (b0f_p, b0f, b0bf),
                                               (b1_p, b1, b1b)):
                        nc.sync.dma_start(src_t[:], src_p[:])
                        for g in range(4):
                            pb = gp.tile([P, HIDDEN], F32, tag="gb",
                                         name=f"pb_{bdst.name}_{g}")
                            nc.tensor.matmul(pb[:], ones[:],
                                             src_t[:, g * HIDDEN:(g + 1) * HIDDEN],
                                             start=True, stop=True)
                            nc.scalar.activation(bdst[:, g], pb[:], AF.Copy)
            with tc.tile_pool(name="pre2", bufs=1) as pre2:
                blin = pre2.tile([1, OUTPUT], F32R, tag="blin")
                nc.sync.dma_start(blin[:], blin_p[:])
                pl = gp.tile([P, OUTPUT], F32, tag="gb", name="pl_bias")
                nc.tensor.matmul(pl[:], ones[:], blin[:], start=True, stop=True)
                nc.scalar.activation(blinb[:], pl[:], AF.Copy)

            ident = wp.tile([P, P], BF16, tag="ident")
            make_identity(nc, ident[:])

            zT = wp.tile([P, KX0, B_LOCAL], BF16, tag="zT_init")
            h0T = wp.tile([P, KH, B_LOCAL], BF16, tag="h0T_init")
            h1T = wp.tile([P, KH, B_LOCAL], BF16, tag="h1T_init")
            c0 = wp.tile([P, HIDDEN], BF16, tag="c0_init")
            c1 = wp.tile([P, HIDDEN], BF16, tag="c1_init")
            nc.sync.dma_start(zT[:], zT_p[:].rearrange("(kc p) b -> p kc b", p=P))
            nc.sync.dma_start(h0T[:], h0T_p[:].rearrange("(kc p) b -> p kc b", p=P))
            nc.sync.dma_start(h1T[:], h1T_p[:].rearrange("(kc p) b -> p kc b", p=P))
            nc.sync.dma_start(c0[:], c0_p[:])
            nc.sync.dma_start(c1[:], c1_p[:])

            def lstm_layer(t, lname, xT, nx, hT, c, wx, wh, bb, b1d):
                """One LSTM layer step. xT: [P, nx, B] bf16 stationary input
                chunks, hT: [P, KH, B] bf16, c: [P, HIDDEN] bf16.
                Returns (hT_new, c_new)."""
                banks = {}
                for j, g in enumerate(GATE_ORDER):
                    ps = gp.tile([P, HIDDEN], F32, tag="gb", name=f"g_{lname}_{t}_{g}")
                    sl = slice(g * HIDDEN, (g + 1) * HIDDEN)
                    if bias_mode == "pe":
                        nc.tensor.matmul(ps[:], ones[:], b1d[:, sl],
                                         start=True, stop=False)
                    elif j % 2 == 0:
                        pi = nc.scalar.activation(ps[:], bb[:, g], AF.Copy)
                    else:
                        pi = nc.vector.tensor_copy(out=ps[:], in_=bb[:, g])
                    if bias_mode != "pe" and depri:
                        pi.ins.bass_priority = PRELOAD_PRIO + 8 * t + j
                    banks[g] = ps
                # recurrent (h) parts first: ready as soon as last step's hT is
                if kmajor:
                    # k-major: 4 consecutive matmuls share one stationary chunk
                    for k in range(KH):
                        for j, g in enumerate(GATE_ORDER):
                            sl = slice(g * HIDDEN, (g + 1) * HIDDEN)
                            nc.tensor.matmul(banks[g][:], hT[:, k], wh[:, k, sl],
                                             start=False, stop=False,
                                             skip_group_check=True)
                    for k in range(nx):
                        for j, g in enumerate(GATE_ORDER):
                            sl = slice(g * HIDDEN, (g + 1) * HIDDEN)
                            nc.tensor.matmul(banks[g][:], xT[:, k], wx[:, k, sl],
                                             start=False, stop=(k == nx - 1),
                                             skip_group_check=True)
                else:
                    for j, g in enumerate(GATE_ORDER):
                        ps = banks[g]
                        sl = slice(g * HIDDEN, (g + 1) * HIDDEN)
                        for k in range(KH):
                            nc.tensor.matmul(ps[:], hT[:, k], wh[:, k, sl],
                                             start=False, stop=False,
                                             skip_group_check=True)
                    # input (x/z) parts second: they wait on the previous stage
                    for j, g in enumerate(GATE_ORDER):
                        ps = banks[g]
                        sl = slice(g * HIDDEN, (g + 1) * HIDDEN)
                        for k in range(nx):
                            nc.tensor.matmul(ps[:], xT[:, k], wx[:, k, sl],
                                             start=False, stop=(k == nx - 1),
                                             skip_group_check=True)

                AD = F32 if act_f32 else BF16
                sf = wk.tile([P, HIDDEN], AD, tag="sf", name=f"sf_{lname}_{t}")
                si = wk.tile([P, HIDDEN], AD, tag="si", name=f"si_{lname}_{t}")
                tg = wk.tile([P, HIDDEN], AD, tag="tg", name=f"tg_{lname}_{t}")
                so = wk.tile([P, HIDDEN], AD, tag="so", name=f"so_{lname}_{t}")
                nc.scalar.activation(sf[:], banks[1][:], AF.Sigmoid)
                nc.scalar.activation(si[:], banks[0][:], AF.Sigmoid)
                nc.scalar.activation(tg[:], banks[2][:], AF.Tanh)
                nc.scalar.activation(so[:], banks[3][:], AF.Sigmoid)

                cn = sp.tile([P, HIDDEN], BF16, tag=f"c_{lname}", name=f"c_{lname}_{t}")
                nc.vector.tensor_mul(out=sf[:], in0=sf[:], in1=c[:])
                nc.vector.tensor_mul(out=si[:], in0=si[:], in1=tg[:])
                nc.vector.tensor_add(out=cn[:], in0=sf[:], in1=si[:])
                nc.scalar.activation(tg[:], cn[:], AF.Tanh)
                hb = wk.tile([P, HIDDEN], BF16, tag="hb", name=f"hb_{lname}_{t}")
                nc.vector.tensor_mul(out=hb[:], in0=so[:], in1=tg[:])

                hTn = sp.tile([P, KH, B_LOCAL], BF16, tag=f"hT_{lname}",
                              name=f"hT_{lname}_{t}")
                if dma_tr:
                    # XBAR DMA transpose straight to SBUF: no PE/DVE involved
                    nc.sync.dma_start_transpose(hTn[:], hb[:])
                    return hTn, cn
                ptr = pp.tile([P, KH, P], BF16, tag="tr", name=f"htr_{lname}_{t}")
                # per-chunk transpose+copy so chunk-k consumers start without
                # waiting for the whole tile
                if chunk_copies:
                    for k in range(KH):
                        nc.tensor.transpose(ptr[:, k], hb[:, k * P:(k + 1) * P],
                                            ident[:])
                        nc.vector.tensor_copy(out=hTn[:, k], in_=ptr[:, k])
                else:
                    for k in range(KH):
                        nc.tensor.transpose(ptr[:, k], hb[:, k * P:(k + 1) * P],
                                            ident[:])
                    nc.vector.tensor_copy(out=hTn[:], in_=ptr[:])
                return hTn, cn

            zT0, h0T0, h1T0, c00, c10 = zT, h0T, h1T, c0, c1

            def time_loop(first_pass):
                ybuf = None
                zT, h0T, h1T, c0, c1 = zT0, h0T0, h1T0, c00, c10
                for t in range(T):
                    if first_pass and t == 0:
                        bb0, b1d0 = b0bf, (b0f if bias_mode == "pe" else None)
                    else:
                        bb0, b1d0 = b0b, (b0 if bias_mode == "pe" else None)
                    h0Tn, c0n = lstm_layer(t, "l0", zT, KX0, h0T, c0, w0x, w0h,
                                           bb0, b1d0)
                    h1Tn, c1n = lstm_layer(t, "l1", h0Tn, KH, h1T, c1, w1x, w1h,
                                           b1b, (b1 if bias_mode == "pe" else None))

                    # fc: z' = h1' @ fc_W.T (bias folded away), feature-major
                    # [z_feat, B]: stationary fc_W.T chunks, moving h1T.
                    zTn = sp.tile([P, KX0, B_LOCAL], BF16, tag="zT", name=f"zT_{t}")
                    pzt = pp.tile([P, KX0, P], F32, tag="tr", name=f"fcT_{t}")
                    for k2 in range(KX0):
                        for k in range(KH):
                            nc.tensor.matmul(pzt[:, k2],
                                             wfc[:, k, k2 * P:(k2 + 1) * P],
                                             h1Tn[:, k],
                                             start=(k == 0), stop=(k == KH - 1))
                    nc.scalar.activation(zTn[:], pzt[:], AF.Copy)

                    # lin composed with fc: out[t] = h1' @ (lin_W@fc_W).T + b';
                    # runs straight off h1Tn, independent of the zTn copyback.
                    plin = pp.tile([P, OUTPUT], F32, tag="tr", name=f"lin_{t}")
                    for k in range(KH):
                        nc.tensor.matmul(plin[:], h1Tn[:, k], wzy[:, k],
                                         start=(k == 0), stop=(k == KH - 1))
                    if t % YB == 0:
                        ybuf = wk.tile([P, YB, OUTPUT], F32, tag="ybuf",
                                       name=f"ybuf_{t}")
                    yi = nc.vector.tensor_add(out=ybuf[:, t % YB], in0=plin[:],
                                              in1=blinb[:])
                    if depri:
                        yi.ins.bass_priority = PRELOAD_PRIO + 8 * t + 6
                    if t % YB == YB - 1 or t == T - 1:
                        n = t % YB + 1
                        nc.gpsimd.dma_start(
                            y_ap[t - n + 1:t + 1].rearrange("t b f -> b t f"),
                            ybuf[:, :n])

                    zT, h0T, h1T, c0, c1 = zTn, h0Tn, h1Tn, c0n, c1n

            if rep == 1:
                time_loop(True)
            else:
                with tc.For_i(0, rep, 1):
                    time_loop(False)

    nc.compile()
    return nc


def make_in_maps(z0, h0, c0, W_ih0, W_hh0, b_ih0, b_hh0,
                 W_ih1, W_hh1, b_ih1, b_hh1, fc_W, fc_b, lin_W, lin_b):
    f = np.float32
    bf = ml_dtypes.bfloat16
    b0_raw = (b_ih0 + b_hh0).astype(f)
    fold = fc_b.astype(f) @ np.asarray(W_ih0, f).T
    shared = {
        "w0x": np.ascontiguousarray(W_ih0.T, dtype=bf),
        "w0h": np.ascontiguousarray(W_hh0.T, dtype=bf),
        "w1x": np.ascontiguousarray(W_ih1.T, dtype=bf),
        "w1h": np.ascontiguousarray(W_hh1.T, dtype=bf),
        "wfc": np.ascontiguousarray(fc_W.T, dtype=bf),
        "wzy": np.ascontiguousarray((np.asarray(lin_W, f) @ np.asarray(fc_W, f)).T,
                                    dtype=bf),
        "bias0": (b0_raw + fold).reshape(1, H4),
        "bias0f": b0_raw.reshape(1, H4),
        "bias1": (b_ih1 + b_hh1).astype(f).reshape(1, H4),
        "blin": (lin_b.astype(f) + fc_b.astype(f) @ np.asarray(lin_W, f).T
                 ).reshape(1, OUTPUT),
        "ones": np.ones((1, B_LOCAL), dtype=f),
    }
    in_maps = []
    for cidx in range(N_CORES):
        sl = slice(cidx * B_LOCAL, (cidx + 1) * B_LOCAL)
        in_maps.append({
            "zT0": np.ascontiguousarray(z0[sl].T, dtype=bf),
            "h0T_l0": np.ascontiguousarray(h0[0, sl].T, dtype=bf),
            "h0T_l1": np.ascontiguousarray(h0[1, sl].T, dtype=bf),
            "c_l0": np.ascontiguousarray(c0[0, sl], dtype=bf),
            "c_l1": np.ascontiguousarray(c0[1, sl], dtype=bf),
            **shared,
        })
    return in_maps


_NC_CACHE = {}


def kernel(z0, h0, c0, W_ih0, W_hh0, b_ih0, b_hh0,
           W_ih1, W_hh1, b_ih1, b_hh1, fc_W, fc_b, lin_W, lin_b, T2):
    T = int(T2)
    if T not in _NC_CACHE:
        _NC_CACHE[T] = build(T)
    nc = _NC_CACHE[T]
    in_maps = make_in_maps(z0, h0, c0, W_ih0, W_hh0, b_ih0, b_hh0,
                           W_ih1, W_hh1, b_ih1, b_hh1, fc_W, fc_b, lin_W, lin_b)
    res = run_bass_kernel_spmd(nc, in_maps, list(range(N_CORES)))
    # per-core y: [T, 128, OUTPUT] -> full [T, 1024, OUTPUT]
    return np.concatenate([r["y"] for r in res.results], axis=1)
